# revision 22
# baseline (speedup 1.0000x reference)
"""Trainium2 Bass kernel for the siamese-kNN classification head.

Reference computation (B=256, N=2000, D=512, C=100):
    scores[b,n] = sigmoid(sum_d w_d * |a[b,d] - S[n,d]| + kb)
    out[b,c]    = (scores @ L)[b,c] / count_c     (0 where count_c == 0)

Strategy
--------
Data-parallel over the batch: core i handles rows 32*i .. 32*i+32, no
collectives.  |x| = 2 relu(x) - x splits the score into a nonlinear slab
relu(A''-S'') (A''= alpha*2|w| (.) a, S''= alpha*2|w| (.) S, bf16) plus an
exact rank-2 f32r correction matmul (kb - w.a_b + (w.S)_n).

The B*N*D slab tensor is the whole cost.  Measured TRN2 engine rates for a
[128,2000] slab chunk:
  DVE tensor_scalar  bf16 out ~760ns   fp8e4 out ~1250ns
  ACT activation     any out  ~1870ns
  PE ingest          bf16 128 elem/cyc; fp8e4 DoubleRow 256 elem/cyc
so slabs are produced in three flavors, tuned so DVE/ACT/PE all finish
together (~82us):
  - 26 rows get one d-chunk-pair as bf16 DVE slabs (PE: 4x [128,500] mm)
  - the other pair of those rows + 6 full rows as fp8 pairs [128,2,2000]
    (DVE x16 pairs, ACT x22), consumed by DoubleRow fp8 matmuls
    ([128,2,250] moving, [128,2,32] sliding sign window) at 2x ingest.
fp8 slabs are alpha=64-scaled so values sit in e4m3's normal range; the
sign windows carry 1/alpha (exact).  End-to-end rel err ~1.5e-2 (fp8
quantization, verified bit-exact against ml_dtypes on host), under the
2e-2 gate.

PE warms up on dummy matmuls during the ~11us DMA cold-start so the
p-state is ramped when the first slab lands.  Tail: sigmoid (ACT) from
PSUM, 16 PE transposes into one PSUM bank, one copy, 16 bf16 label
matmuls, 1/count scale (host-prepared divide-no-nan), DMA out.
"""

import sys

for _p in ("/opt/trn_rl_repo", "/root/.axon_site/_ro/trn_rl_repo"):
    if _p not in sys.path:
        sys.path.append(_p)

import numpy as np

B, N, D, C = 256, 2000, 512, 100
NP = 2048                  # label rows padded to 16 full chunks
NCORES = 8
BSH = B // NCORES          # 32 batch rows per core
DCH = D // 128             # 4 d-chunks
NSEG = 4                   # PSUM free-dim segments
SEG = N // NSEG            # 500
HSEG = SEG // 2            # 250 (DoubleRow moving limit: 2*250 <= 512)
NLAB = NP // 128           # 16 label chunks
ALPHA = 64.0               # fp8 range pre-scale (exact power of 2)

F8_SPLIT = 1               # fp8 matmuls per psc segment (1 -> [128,2,500])
F8C = SEG // F8_SPLIT
KTAIL = 6                  # last units emitted segment-major (early psc stops)

# ---- producer assignment ----
# rows 0..20: mixed -- one chunk-pair bf16 (DVE), the other fp8.
#   even b: fp8 pair = 0; odd b: fp8 pair = 1.
# rows 21..31: both pairs fp8 (21-26 DVE, 27-31 ACT).
_MIX_DVE = frozenset(range(10))   # mixed rows whose fp8 pair runs on DVE
_FULL_DVE = (21, 22, 23, 24, 25, 26)
_FULL_ACT = (27, 28, 29, 30, 31)
N_MIXED = 21

_CACHE = {}


def _f8_units():
    """Canonical order of the fp8 (prod, pair, b) units; index = dense
    weight-window slot (dual-fp8 ldweights needs contiguous aligned
    [128,2,32] weights, so windows are materialized per unit)."""
    order = []
    for p in (0, 1):
        f8_rows = [b for b in range(N_MIXED) if (b % 2 == 0) == (p == 0)]
        for b in [x for x in f8_rows if x in _MIX_DVE] + list(_FULL_DVE):
            order.append(("dve", p, b))
        for b in [x for x in f8_rows if x not in _MIX_DVE] + list(_FULL_ACT):
            order.append(("act", p, b))
    return order


NF8 = len(_f8_units())


def _plan():
    """Static schedule: producer instruction order + PE consumption order.

    Items: dict(kind='bfs', ch, b) one bf16 chunk-slab, or
           dict(kind='f8p', pair, b, prod) one fp8 chunk-pair.
    Emission order of PE matmuls = estimated completion order, so the
    in-order PE never waits on a later-finishing producer while an
    earlier slab sits ready.
    """
    CH_LAND = [9900.0, 10900.0, 11900.0, 12900.0]
    AB_LAND = 9300.0
    T_BF, T_F8H, T_AH = 760.0, 1150.0, 1870.0

    def interleave(la, lb):
        out, ia, ib = [], 0, 0
        while ia < len(la) or ib < len(lb):
            if ib >= len(lb) or (ia < len(la) and ia * len(lb) <= ib * len(la)):
                out.append(la[ia]); ia += 1
            else:
                out.append(lb[ib]); ib += 1
        return out

    items, dve_prog, act_prog = [], [], []
    for p in (0, 1):
        ch0, ch1 = 2 * p, 2 * p + 1
        bf_rows = [b for b in range(N_MIXED) if (b % 2 == 0) == (p == 1)]
        f8_rows = [b for b in range(N_MIXED) if (b % 2 == 0) == (p == 0)]
        f8_dve = [b for b in f8_rows if b in _MIX_DVE] + list(_FULL_DVE)
        f8_act = [b for b in f8_rows if b not in _MIX_DVE] + list(_FULL_ACT)

        s0 = [dict(kind="bfs", ch=ch0, b=b) for b in bf_rows]
        s1 = [dict(kind="bfs", ch=ch1, b=b) for b in bf_rows]
        fv = [dict(kind="f8p", pair=p, b=b, prod="dve") for b in f8_dve]
        fa = [dict(kind="f8p", pair=p, b=b, prod="act") for b in f8_act]
        items += s0 + s1 + fv + fa

        dve_prog += [("bfs", u) for u in s0]
        for entry in interleave([("bfs", u) for u in s1],
                                [("f8p", u) for u in fv]):
            if entry[0] == "bfs":
                dve_prog.append(entry)
            else:
                dve_prog.append(("f8h", entry[1], 0))
                dve_prog.append(("f8h", entry[1], 1))
        for u in fa:
            act_prog.append(("f8h", u, 0))
            act_prog.append(("f8h", u, 1))

    def sim(prog, t_bf, t_half):
        clock = 0.0
        for entry in prog:
            if entry[0] == "bfs":
                u = entry[1]
                clock = max(clock, CH_LAND[u["ch"]], AB_LAND) + t_bf
                u["done"] = clock
            else:
                _, u, h = entry
                clock = max(clock, CH_LAND[2 * u["pair"] + h], AB_LAND) + t_half
                if h == 1:
                    u["done"] = clock

    sim(dve_prog, T_BF, T_F8H)
    sim(act_prog, 0.0, T_AH)
    widx = {u: j for j, u in enumerate(_f8_units())}
    for u in items:
        if u["kind"] == "f8p":
            u["widx"] = widx[(u["prod"], u["pair"], u["b"])]
    pe_order = sorted(items, key=lambda u: u["done"])
    assert pe_order[0]["kind"] == "bfs" and pe_order[0]["ch"] == 0
    return dve_prog, act_prog, pe_order


def _split_multi_waits(nc):
    """TRN2 TPB instructions encode at most ONE semaphore wait; split extras
    into single-wait NOPs directly before the instruction (same engine)."""
    from concourse import mybir

    for fn in nc.m.functions:
        for bb in fn.blocks:
            out = []
            for inst in bb.instructions:
                si = inst.sync_info
                if si is not None and si.on_wait and len(si.on_wait) > 1:
                    waits = list(si.on_wait)
                    for j, w in enumerate(waits[:-1]):
                        out.append(mybir.InstNoOp(
                            name=f"{inst.name}-sw{j}", engine=inst.engine,
                            sync_info=mybir.SyncInfo(on_wait=[w], on_update=[]),
                            ins=[], outs=[]))
                    inst.sync_info = mybir.SyncInfo(
                        on_wait=[waits[-1]], on_update=list(si.on_update))
                out.append(inst)
            bb.instructions = out


def _build_nc():
    import concourse.bass as bass
    import concourse.tile as tile
    from concourse import mybir

    f32 = mybir.dt.float32
    f32r = mybir.dt.float32r
    bf16 = mybir.dt.bfloat16
    f8 = mybir.dt.float8e4
    nc = bass.Bass()

    s2t_d = nc.declare_dram_parameter("s2t", [D, N], bf16, isOutput=False)
    a2t_d = nc.declare_dram_parameter("a2t", [DCH, 128, BSH], f32, isOutput=False)
    sgnb_d = nc.declare_dram_parameter("sgnb", [128, DCH, 63], bf16, isOutput=False)
    sgw_d = nc.declare_dram_parameter("sgw", [128, NF8, 2, 32], f8, isOutput=False)
    cc_d = nc.declare_dram_parameter("cc", [2, N + BSH], f32r, isOutput=False)
    labsid_d = nc.declare_dram_parameter("labsid", [128, NLAB * C + 32],
                                         bf16, isOutput=False)
    recb_d = nc.declare_dram_parameter("recb", [BSH, C], f32, isOutput=False)
    out_d = nc.declare_dram_parameter("out", [BSH, C], f32, isOutput=True)

    dve_prog, act_prog, pe_order = _plan()
    sub = None  # set after mybir import below

    with tile.TileContext(nc) as tc:
        with (
            tc.tile_pool(name="const", bufs=1) as const,
            tc.tile_pool(name="dslab", bufs=6) as dpool,
            tc.tile_pool(name="vpair", bufs=3) as vpool,
            tc.tile_pool(name="apair", bufs=3) as apool,
            tc.tile_pool(name="bank", bufs=8, space="PSUM") as bankp,
        ):
            # ---- DMAs: s2t chunks serial on the Sync queue; everything else
            # issued in parallel from the (otherwise idle) GpSimd queue.
            s2t0 = const.tile([128, N], bf16, name="s2t0", tag="s2t0")
            nc.sync.dma_start(s2t0[:], s2t_d[0:128, :])
            s2t123 = const.tile([128, 3, N], bf16, name="s2t123", tag="s2t123")
            nc.sync.dma_start(s2t123[:, 0, :], s2t_d[128:256, :])
            nc.sync.dma_start(s2t123[:, 1, :], s2t_d[256:384, :])
            nc.sync.dma_start(s2t123[:, 2, :], s2t_d[384:512, :])
            a2t = const.tile([128, DCH * BSH], f32, name="a2t", tag="a2t")
            nc.gpsimd.dma_start(
                a2t[:].rearrange("p (c b) -> p c b", c=DCH),
                a2t_d[:].rearrange("c p b -> p c b"),
            )
            sgnb = const.tile([128, DCH, 63], bf16, name="sgnb", tag="sgnb")
            nc.gpsimd.dma_start(sgnb[:], sgnb_d[:])
            sgw = const.tile([128, NF8, 2, 32], f8, name="sgw", tag="sgw")
            nc.gpsimd.dma_start(sgw[:], sgw_d[:])
            cc = const.tile([2, N + BSH], f32r, name="cc", tag="cc")
            nc.gpsimd.dma_start(cc[:], cc_d[:])
            labsid = const.tile([128, NLAB * C + 32], bf16,
                               name="labsid", tag="labsid")
            nc.gpsimd.dma_start(labsid[:], labsid_d[:])
            recb = const.tile([BSH, C], f32, name="recb", tag="recb")
            nc.gpsimd.dma_start(recb[:], recb_d[:])

            s2t = [s2t0] + [s2t123[:, k, :] for k in range(3)]
            ident = labsid[0:32, NLAB * C : NLAB * C + 32]

            sub_op = mybir.AluOpType.subtract
            min_op = mybir.AluOpType.min
            relu = mybir.ActivationFunctionType.Relu

            psc = [
                bankp.tile([BSH, SEG], f32, name=f"psc{s}", tag="bank")
                for s in range(NSEG)
            ]

            # ---- PE p-state warmup while DMAs run
            dummy_sb = const.tile([128, 512], bf16, name="dummy", tag="dummy")
            nc.vector.memset(dummy_sb[:], 0.0)
            dummy_ps = bankp.tile([2, 512], f32, name="dummy_ps", tag="bank")
            for _ in range(11):
                nc.tensor.matmul(
                    dummy_ps[:], dummy_sb[:, 0:2], dummy_sb[:],
                    start=True, stop=True, skip_group_check=True,
                )

            # ---- producers (DVE / ACT program order from the plan)
            for entry in dve_prog:
                if entry[0] == "bfs":
                    u = entry[1]
                    slab = dpool.tile([128, N], bf16, name="dslab", tag="dslab")
                    nc.vector.tensor_scalar(
                        slab[:], s2t[u["ch"]],
                        a2t[:, u["ch"] * BSH + u["b"] : u["ch"] * BSH + u["b"] + 1],
                        0.0, sub_op, min_op,
                    )
                    u["ap"] = slab
                else:
                    _, u, h = entry
                    if h == 0:
                        u["ap"] = vpool.tile([128, NSEG * F8_SPLIT, 2, F8C], f8,
                                             name="vpair", tag="vpair")
                    ch = 2 * u["pair"] + h
                    nc.vector.tensor_scalar(
                        u["ap"][:, :, h, :], s2t[ch],
                        a2t[:, ch * BSH + u["b"] : ch * BSH + u["b"] + 1],
                        0.0, sub_op, min_op,
                    )
            for entry in act_prog:
                _, u, h = entry
                if h == 0:
                    u["ap"] = apool.tile([128, NSEG * F8_SPLIT, 2, F8C], f8,
                                         name="apair", tag="apair")
                ch = 2 * u["pair"] + h
                nc.scalar.activation(
                    u["ap"][:, :, h, :], s2t[ch], relu,
                    bias=a2t[:, ch * BSH + u["b"] : ch * BSH + u["b"] + 1],
                    scale=-1.0,
                )

            # ---- PE stream in estimated completion order
            def unit_seg_mms(u, s, first, stop):
                b = u["b"]
                if u["kind"] == "bfs":
                    nc.tensor.matmul(
                        psc[s][:], sgnb[:, u["ch"], 31 - b : 63 - b],
                        u["ap"][:, SEG * s : SEG * (s + 1)],
                        start=first, stop=stop,
                        skip_group_check=not first,
                    )
                else:
                    for j in range(F8_SPLIT):
                        nc.tensor.matmul(
                            psc[s][:, F8C * j : F8C * (j + 1)],
                            sgw[:, u["widx"], :, :],
                            u["ap"][:, F8_SPLIT * s + j, :, :],
                            start=False, stop=(stop and j == F8_SPLIT - 1),
                            perf_mode=mybir.MatmulPerfMode.DoubleRow,
                            skip_group_check=True,
                        )

            last_idx = len(pe_order) - 1
            for idx, u in enumerate(pe_order):
                for s in range(NSEG):
                    unit_seg_mms(u, s, first=(idx == 0), stop=(idx == last_idx))
                if idx == 6:
                    # exact rank-2 correction: kb - w.a_b + (w.S)_n
                    for s in range(NSEG):
                        nc.tensor.matmul(
                            psc[s][:], cc[:, N : N + BSH],
                            cc[:, SEG * s : SEG * (s + 1)],
                            start=False, stop=False, skip_group_check=True,
                        )

            # ---- sigmoid (PSUM -> SBUF, bf16) ----
            ssig = const.tile([BSH, N], bf16, name="ssig", tag="ssig")
            for s in range(NSEG):
                nc.scalar.activation(
                    ssig[:, SEG * s : SEG * (s + 1)], psc[s][:],
                    mybir.ActivationFunctionType.Sigmoid,
                )

            # ---- transposes into ONE PSUM bank, quartet-pipelined copy +
            # label matmuls (copy chunk j frees transposes 4j..4j+3)
            tpall = bankp.tile([128, NLAB * BSH], bf16, name="tpall", tag="bank")
            sct = const.tile([128, NLAB * BSH], bf16, name="sct", tag="sct")
            out_ps = bankp.tile([BSH, C], f32, name="out_ps", tag="bank")
            def label_mms(q):
                for k in range(4 * q, 4 * q + 4):
                    pk = min(128, N - 128 * k)
                    nc.tensor.matmul(
                        out_ps[:], sct[:pk, BSH * k : BSH * k + BSH],
                        labsid[:pk, C * k : C * (k + 1)],
                        start=(k == 0), stop=(k == NLAB - 1),
                    )

            for q in range(4):
                for k in range(4 * q, 4 * q + 4):
                    pk = min(128, N - 128 * k)
                    nc.tensor.transpose(
                        tpall[:pk, BSH * k : BSH * k + BSH],
                        ssig[:, 128 * k : 128 * k + pk], ident,
                    )
                nc.vector.tensor_copy(
                    sct[:, BSH * 4 * q : BSH * 4 * (q + 1)],
                    tpall[:, BSH * 4 * q : BSH * 4 * (q + 1)],
                )
                if q > 0:
                    label_mms(q - 1)
            label_mms(3)

            # ---- divide by counts, write out (GpSimd DMA queue: idle) ----
            out_s = const.tile([BSH, C], f32, name="out_s", tag="out_s")
            nc.vector.tensor_mul(out_s[:], out_ps[:], recb[:])
            nc.gpsimd.dma_start(out_d[:], out_s[:])

    _split_multi_waits(nc)
    return nc


def _prep_host(inputs, support_tensors, support_labels, kernel_w, kernel_b):
    import ml_dtypes

    bf16 = ml_dtypes.bfloat16
    f8 = ml_dtypes.float8_e4m3
    a = np.asarray(inputs, dtype=np.float32)
    S = np.asarray(support_tensors, dtype=np.float32)
    L = np.asarray(support_labels, dtype=np.float32)
    w = np.asarray(kernel_w, dtype=np.float32)
    kb = np.float32(np.asarray(kernel_b, dtype=np.float32))

    aw = ALPHA * 2.0 * np.abs(w)
    sgn = np.sign(w).astype(np.float32)
    s2t = np.ascontiguousarray((S * aw[None, :]).T).astype(bf16)   # [D, N]
    wS = (S @ w).astype(np.float32)                                # [N]
    wa = (a @ w).astype(np.float32)                                # [B]
    a2 = a * aw[None, :]                                           # [B, D]

    sgn_chunks = sgn.reshape(DCH, 128).T                           # [128, DCH]
    # bf16 sliding-window sign tiles (negative slabs): col 31 = -sgn/alpha
    sgnb = np.zeros((128, DCH, 63), dtype=np.float32)
    sgnb[:, :, 31] = -sgn_chunks / ALPHA
    # dense fp8 weight windows, one [128,2,32] slot per fp8 unit
    sgw = np.zeros((128, NF8, 2, 32), dtype=np.float32)
    for j, (prod, p, b) in enumerate(_f8_units()):
        pol = -1.0 if prod == "dve" else 1.0
        for i in range(2):
            sgw[:, j, i, b] = pol * sgn_chunks[:, 2 * p + i] / ALPHA
    sgw = sgw.astype(f8)

    labp = np.zeros((NP, C), dtype=np.float32)
    labp[:N] = L
    labp = labp.reshape(NLAB, 128, C).transpose(1, 0, 2).reshape(128, NLAB * C)
    labsid = np.zeros((128, NLAB * C + 32), dtype=np.float32)
    labsid[:, : NLAB * C] = labp
    labsid[0:32, NLAB * C : NLAB * C + 32] = np.eye(32, dtype=np.float32)
    labsid = labsid.astype(bf16)

    counts = L.sum(axis=0)
    recip = np.where(counts != 0, 1.0 / np.maximum(counts, 1e-30), 0.0)
    recb = np.broadcast_to(recip.astype(np.float32), (BSH, C)).copy()

    shared = {
        "s2t": s2t, "sgnb": sgnb.astype(bf16), "sgw": sgw,
        "labsid": labsid, "recb": recb,
    }
    in_maps = []
    for c in range(NCORES):
        rows = slice(BSH * c, BSH * (c + 1))
        a2t_c = np.ascontiguousarray(
            a2[rows].T.reshape(DCH, 128, BSH))                     # [DCH,128,BSH]
        cc = np.zeros((2, N + BSH), dtype=np.float32)
        cc[0, :N] = 1.0
        cc[1, :N] = wS
        cc[0, N:] = kb - wa[rows]
        cc[1, N:] = 1.0
        in_maps.append(dict(shared, a2t=a2t_c, cc=cc))
    return in_maps


def kernel(**inputs) -> np.ndarray:
    from concourse.bass_utils import run_bass_kernel_spmd

    if "nc" not in _CACHE:
        _CACHE["nc"] = _build_nc()
    nc = _CACHE["nc"]

    in_maps = _prep_host(
        inputs["inputs"], inputs["support_tensors"], inputs["support_labels"],
        inputs["kernel_w"], inputs["kernel_b"],
    )
    res = run_bass_kernel_spmd(nc, in_maps, list(range(NCORES)))
    return np.concatenate([res.results[i]["out"] for i in range(NCORES)], axis=0)


# revision 27
# speedup vs baseline: 1.0060x; 1.0060x over previous
"""Trainium2 Bass kernel for the siamese-kNN classification head.

Reference computation (B=256, N=2000, D=512, C=100):
    scores[b,n] = sigmoid(sum_d w_d * |a[b,d] - S[n,d]| + kb)
    out[b,c]    = (scores @ L)[b,c] / count_c     (0 where count_c == 0)

Strategy
--------
Data-parallel over the batch: core i handles rows 32*i .. 32*i+32, no
collectives.  |x| = 2 relu(x) - x splits the score into a nonlinear slab
relu(A''-S'') (A''= alpha*2|w| (.) a, S''= alpha*2|w| (.) S, bf16) plus an
exact rank-2 f32r correction matmul (kb - w.a_b + (w.S)_n).

The B*N*D slab tensor is the whole cost.  Measured TRN2 engine rates for a
[128,2000] slab chunk:
  DVE tensor_scalar  bf16 out ~760ns   fp8e4 out ~1250ns
  ACT activation     any out  ~1870ns
  PE ingest          bf16 128 elem/cyc; fp8e4 DoubleRow 256 elem/cyc
so slabs are produced in three flavors, tuned so DVE/ACT/PE all finish
together (~82us):
  - 26 rows get one d-chunk-pair as bf16 DVE slabs (PE: 4x [128,500] mm)
  - the other pair of those rows + 6 full rows as fp8 pairs [128,2,2000]
    (DVE x16 pairs, ACT x22), consumed by DoubleRow fp8 matmuls
    ([128,2,250] moving, [128,2,32] sliding sign window) at 2x ingest.
fp8 slabs are alpha=64-scaled so values sit in e4m3's normal range; the
sign windows carry 1/alpha (exact).  End-to-end rel err ~1.5e-2 (fp8
quantization, verified bit-exact against ml_dtypes on host), under the
2e-2 gate.

PE warms up on dummy matmuls during the ~11us DMA cold-start so the
p-state is ramped when the first slab lands.  Tail: sigmoid (ACT) from
PSUM, 16 PE transposes into one PSUM bank, one copy, 16 bf16 label
matmuls, 1/count scale (host-prepared divide-no-nan), DMA out.
"""

import sys

for _p in ("/opt/trn_rl_repo", "/root/.axon_site/_ro/trn_rl_repo"):
    if _p not in sys.path:
        sys.path.append(_p)

import numpy as np

B, N, D, C = 256, 2000, 512, 100
NP = 2048                  # label rows padded to 16 full chunks
NCORES = 8
BSH = B // NCORES          # 32 batch rows per core
DCH = D // 128             # 4 d-chunks
NSEG = 4                   # PSUM free-dim segments
SEG = N // NSEG            # 500
HSEG = SEG // 2            # 250 (DoubleRow moving limit: 2*250 <= 512)
NLAB = NP // 128           # 16 label chunks
ALPHA = 64.0               # fp8 range pre-scale (exact power of 2)

F8_SPLIT = 1               # fp8 matmuls per psc segment (1 -> [128,2,500])
F8C = SEG // F8_SPLIT
KTAIL = 6                  # last units emitted segment-major (early psc stops)

# ---- producer assignment ----
# rows 0..17: mixed -- one chunk-pair bf16 (DVE), the other fp8.
#   even b: fp8 pair = 0; odd b: fp8 pair = 1.
# rows 18..31: both pairs fp8 (18-25 DVE, 26-31 ACT).
_MIX_DVE = frozenset(range(9))    # mixed rows whose fp8 pair runs on DVE
_FULL_DVE = (18, 19, 20, 21, 22, 23, 24, 25)
_FULL_ACT = (26, 27, 28, 29, 30, 31)
N_MIXED = 18

_CACHE = {}


def _f8_units():
    """Canonical order of the fp8 (prod, pair, b) units; index = dense
    weight-window slot (dual-fp8 ldweights needs contiguous aligned
    [128,2,32] weights, so windows are materialized per unit)."""
    order = []
    for p in (0, 1):
        f8_rows = [b for b in range(N_MIXED) if (b % 2 == 0) == (p == 0)]
        for b in [x for x in f8_rows if x in _MIX_DVE] + list(_FULL_DVE):
            order.append(("dve", p, b))
        for b in [x for x in f8_rows if x not in _MIX_DVE] + list(_FULL_ACT):
            order.append(("act", p, b))
    return order


NF8 = len(_f8_units())


def _plan():
    """Static schedule: producer instruction order + PE consumption order.

    Items: dict(kind='bfs', ch, b) one bf16 chunk-slab, or
           dict(kind='f8p', pair, b, prod) one fp8 chunk-pair.
    Emission order of PE matmuls = estimated completion order, so the
    in-order PE never waits on a later-finishing producer while an
    earlier slab sits ready.
    """
    CH_LAND = [11000.0, 12700.0, 14000.0, 15400.0]
    AB_LAND = 11000.0
    T_BF, T_F8H, T_AH = 760.0, 1150.0, 1870.0

    def interleave(la, lb):
        out, ia, ib = [], 0, 0
        while ia < len(la) or ib < len(lb):
            if ib >= len(lb) or (ia < len(la) and ia * len(lb) <= ib * len(la)):
                out.append(la[ia]); ia += 1
            else:
                out.append(lb[ib]); ib += 1
        return out

    items, dve_prog, act_prog = [], [], []
    for p in (0, 1):
        ch0, ch1 = 2 * p, 2 * p + 1
        bf_rows = [b for b in range(N_MIXED) if (b % 2 == 0) == (p == 1)]
        f8_rows = [b for b in range(N_MIXED) if (b % 2 == 0) == (p == 0)]
        f8_dve = [b for b in f8_rows if b in _MIX_DVE] + list(_FULL_DVE)
        f8_act = [b for b in f8_rows if b not in _MIX_DVE] + list(_FULL_ACT)

        s0 = [dict(kind="bfs", ch=ch0, b=b) for b in bf_rows]
        s1 = [dict(kind="bfs", ch=ch1, b=b) for b in bf_rows]
        fv = [dict(kind="f8p", pair=p, b=b, prod="dve") for b in f8_dve]
        fa = [dict(kind="f8p", pair=p, b=b, prod="act") for b in f8_act]
        items += s0 + s1 + fv + fa

        dve_prog += [("bfs", u) for u in s0]
        for entry in interleave([("bfs", u) for u in s1],
                                [("f8p", u) for u in fv]):
            if entry[0] == "bfs":
                dve_prog.append(entry)
            else:
                dve_prog.append(("f8h", entry[1], 0))
                dve_prog.append(("f8h", entry[1], 1))
        for u in fa:
            act_prog.append(("f8h", u, 0))
            act_prog.append(("f8h", u, 1))

    def sim(prog, t_bf, t_half):
        clock = 0.0
        for entry in prog:
            if entry[0] == "bfs":
                u = entry[1]
                clock = max(clock, CH_LAND[u["ch"]], AB_LAND) + t_bf
                u["done"] = clock
            else:
                _, u, h = entry
                clock = max(clock, CH_LAND[2 * u["pair"] + h], AB_LAND) + t_half
                if h == 1:
                    u["done"] = clock

    sim(dve_prog, T_BF, T_F8H)
    sim(act_prog, 0.0, T_AH)
    widx = {u: j for j, u in enumerate(_f8_units())}
    for u in items:
        if u["kind"] == "f8p":
            u["widx"] = widx[(u["prod"], u["pair"], u["b"])]
    pe_order = sorted(items, key=lambda u: u["done"])
    assert pe_order[0]["kind"] == "bfs" and pe_order[0]["ch"] == 0
    return dve_prog, act_prog, pe_order


def _split_multi_waits(nc):
    """TRN2 TPB instructions encode at most ONE semaphore wait; split extras
    into single-wait NOPs directly before the instruction (same engine)."""
    from concourse import mybir

    for fn in nc.m.functions:
        for bb in fn.blocks:
            out = []
            for inst in bb.instructions:
                si = inst.sync_info
                if si is not None and si.on_wait and len(si.on_wait) > 1:
                    waits = list(si.on_wait)
                    for j, w in enumerate(waits[:-1]):
                        out.append(mybir.InstNoOp(
                            name=f"{inst.name}-sw{j}", engine=inst.engine,
                            sync_info=mybir.SyncInfo(on_wait=[w], on_update=[]),
                            ins=[], outs=[]))
                    inst.sync_info = mybir.SyncInfo(
                        on_wait=[waits[-1]], on_update=list(si.on_update))
                out.append(inst)
            bb.instructions = out


def _build_nc():
    import concourse.bass as bass
    import concourse.tile as tile
    from concourse import mybir

    f32 = mybir.dt.float32
    f32r = mybir.dt.float32r
    bf16 = mybir.dt.bfloat16
    f8 = mybir.dt.float8e4
    nc = bass.Bass()

    s2t_d = nc.declare_dram_parameter("s2t", [D, N], bf16, isOutput=False)
    a2t_d = nc.declare_dram_parameter("a2t", [DCH, 128, BSH], f32, isOutput=False)
    sgnb_d = nc.declare_dram_parameter("sgnb", [128, DCH, 63], bf16, isOutput=False)
    sgw_d = nc.declare_dram_parameter("sgw", [128, NF8, 2, 32], f8, isOutput=False)
    cc_d = nc.declare_dram_parameter("cc", [2, N + BSH], f32r, isOutput=False)
    labsid_d = nc.declare_dram_parameter("labsid", [128, NLAB * C + 32],
                                         bf16, isOutput=False)
    recb_d = nc.declare_dram_parameter("recb", [BSH, C], f32, isOutput=False)
    out_d = nc.declare_dram_parameter("out", [BSH, C], f32, isOutput=True)

    dve_prog, act_prog, pe_order = _plan()
    sub = None  # set after mybir import below

    with tile.TileContext(nc) as tc:
        with (
            tc.tile_pool(name="const", bufs=1) as const,
            tc.tile_pool(name="dslab", bufs=6) as dpool,
            tc.tile_pool(name="vpair", bufs=3) as vpool,
            tc.tile_pool(name="apair", bufs=3) as apool,
            tc.tile_pool(name="bank", bufs=8, space="PSUM") as bankp,
        ):
            # ---- DMAs: s2t chunks (big, latency-critical) serial on the
            # Sync queue -- first chunk split in halves so the first slab
            # starts ~1us earlier.  Small constants go on the (otherwise
            # idle) GpSimd queue in parallel; big late-needed tensors (sgw,
            # labsid) follow the chunks on Sync to keep the first-chunk
            # window free of bandwidth contention.
            s2t0 = const.tile([128, N], bf16, name="s2t0", tag="s2t0")
            nc.sync.dma_start(s2t0[:, 0 : N // 2], s2t_d[0:128, 0 : N // 2])
            nc.sync.dma_start(s2t0[:, N // 2 : N], s2t_d[0:128, N // 2 : N])
            s2t123 = const.tile([128, 3, N], bf16, name="s2t123", tag="s2t123")
            nc.sync.dma_start(s2t123[:, 0, :], s2t_d[128:256, :])
            nc.sync.dma_start(s2t123[:, 1, :], s2t_d[256:384, :])
            nc.sync.dma_start(s2t123[:, 2, :], s2t_d[384:512, :])
            sgw = const.tile([128, NF8, 2, 32], f8, name="sgw", tag="sgw")
            nc.sync.dma_start(sgw[:], sgw_d[:])
            labsid = const.tile([128, NLAB * C + 32], bf16,
                               name="labsid", tag="labsid")
            nc.sync.dma_start(labsid[:], labsid_d[:])
            a2t = const.tile([128, DCH * BSH], f32, name="a2t", tag="a2t")
            nc.gpsimd.dma_start(
                a2t[:].rearrange("p (c b) -> p c b", c=DCH),
                a2t_d[:].rearrange("c p b -> p c b"),
            )
            sgnb = const.tile([128, DCH, 63], bf16, name="sgnb", tag="sgnb")
            nc.gpsimd.dma_start(sgnb[:], sgnb_d[:])
            cc = const.tile([2, N + BSH], f32r, name="cc", tag="cc")
            nc.gpsimd.dma_start(cc[:], cc_d[:])
            recb = const.tile([BSH, C], f32, name="recb", tag="recb")
            nc.gpsimd.dma_start(recb[:], recb_d[:])

            s2t = [s2t0] + [s2t123[:, k, :] for k in range(3)]
            ident = labsid[0:32, NLAB * C : NLAB * C + 32]

            sub_op = mybir.AluOpType.subtract
            min_op = mybir.AluOpType.min
            relu = mybir.ActivationFunctionType.Relu

            psc = [
                bankp.tile([BSH, SEG], f32, name=f"psc{s}", tag="bank")
                for s in range(NSEG)
            ]

            # ---- PE p-state warmup while DMAs run
            dummy_sb = const.tile([128, 512], bf16, name="dummy", tag="dummy")
            nc.vector.memset(dummy_sb[:], 0.0)
            dummy_ps = bankp.tile([2, 512], f32, name="dummy_ps", tag="bank")
            for _ in range(12):
                nc.tensor.matmul(
                    dummy_ps[:], dummy_sb[:, 0:2], dummy_sb[:],
                    start=True, stop=True, skip_group_check=True,
                )

            # ---- producers (DVE / ACT program order from the plan)
            n_bfs_seen = 0
            for entry in dve_prog:
                if entry[0] == "bfs":
                    u = entry[1]
                    slab = dpool.tile([128, N], bf16, name="dslab", tag="dslab")
                    col = u["ch"] * BSH + u["b"]
                    if n_bfs_seen < 2:
                        # first slabs chase the split s2t0 halves
                        for lo, hi in ((0, N // 2), (N // 2, N)):
                            nc.vector.tensor_scalar(
                                slab[:, lo:hi], s2t[u["ch"]][:, lo:hi],
                                a2t[:, col : col + 1], 0.0, sub_op, min_op,
                            )
                    else:
                        nc.vector.tensor_scalar(
                            slab[:], s2t[u["ch"]], a2t[:, col : col + 1],
                            0.0, sub_op, min_op,
                        )
                    n_bfs_seen += 1
                    u["ap"] = slab
                else:
                    _, u, h = entry
                    if h == 0:
                        u["ap"] = vpool.tile([128, NSEG * F8_SPLIT, 2, F8C], f8,
                                             name="vpair", tag="vpair")
                    ch = 2 * u["pair"] + h
                    nc.vector.tensor_scalar(
                        u["ap"][:, :, h, :], s2t[ch],
                        a2t[:, ch * BSH + u["b"] : ch * BSH + u["b"] + 1],
                        0.0, sub_op, min_op,
                    )
            for entry in act_prog:
                _, u, h = entry
                if h == 0:
                    u["ap"] = apool.tile([128, NSEG * F8_SPLIT, 2, F8C], f8,
                                         name="apair", tag="apair")
                ch = 2 * u["pair"] + h
                nc.scalar.activation(
                    u["ap"][:, :, h, :], s2t[ch], relu,
                    bias=a2t[:, ch * BSH + u["b"] : ch * BSH + u["b"] + 1],
                    scale=-1.0,
                )

            # ---- PE stream in estimated completion order
            def unit_seg_mms(u, s, first, stop):
                b = u["b"]
                if u["kind"] == "bfs":
                    nc.tensor.matmul(
                        psc[s][:], sgnb[:, u["ch"], 31 - b : 63 - b],
                        u["ap"][:, SEG * s : SEG * (s + 1)],
                        start=first, stop=stop,
                        skip_group_check=not first,
                    )
                else:
                    for j in range(F8_SPLIT):
                        nc.tensor.matmul(
                            psc[s][:, F8C * j : F8C * (j + 1)],
                            sgw[:, u["widx"], :, :],
                            u["ap"][:, F8_SPLIT * s + j, :, :],
                            start=False, stop=(stop and j == F8_SPLIT - 1),
                            perf_mode=mybir.MatmulPerfMode.DoubleRow,
                            skip_group_check=True,
                        )

            last_idx = len(pe_order) - 1
            for idx, u in enumerate(pe_order):
                for s in range(NSEG):
                    unit_seg_mms(u, s, first=(idx == 0), stop=(idx == last_idx))
                if idx == 6:
                    # exact rank-2 correction: kb - w.a_b + (w.S)_n
                    for s in range(NSEG):
                        nc.tensor.matmul(
                            psc[s][:], cc[:, N : N + BSH],
                            cc[:, SEG * s : SEG * (s + 1)],
                            start=False, stop=False, skip_group_check=True,
                        )

            # ---- sigmoid (PSUM -> SBUF, bf16) ----
            ssig = const.tile([BSH, N], bf16, name="ssig", tag="ssig")
            for s in range(NSEG):
                nc.scalar.activation(
                    ssig[:, SEG * s : SEG * (s + 1)], psc[s][:],
                    mybir.ActivationFunctionType.Sigmoid,
                )

            # ---- transposes into ONE PSUM bank, quartet-pipelined copy +
            # label matmuls (copy chunk j frees transposes 4j..4j+3)
            tpall = bankp.tile([128, NLAB * BSH], bf16, name="tpall", tag="bank")
            sct = const.tile([128, NLAB * BSH], bf16, name="sct", tag="sct")
            out_ps = bankp.tile([BSH, C], f32, name="out_ps", tag="bank")
            def label_mms(q):
                for k in range(4 * q, 4 * q + 4):
                    pk = min(128, N - 128 * k)
                    nc.tensor.matmul(
                        out_ps[:], sct[:pk, BSH * k : BSH * k + BSH],
                        labsid[:pk, C * k : C * (k + 1)],
                        start=(k == 0), stop=(k == NLAB - 1),
                    )

            for q in range(4):
                for k in range(4 * q, 4 * q + 4):
                    pk = min(128, N - 128 * k)
                    nc.tensor.transpose(
                        tpall[:pk, BSH * k : BSH * k + BSH],
                        ssig[:, 128 * k : 128 * k + pk], ident,
                    )
                nc.vector.tensor_copy(
                    sct[:, BSH * 4 * q : BSH * 4 * (q + 1)],
                    tpall[:, BSH * 4 * q : BSH * 4 * (q + 1)],
                )
                if q > 0:
                    label_mms(q - 1)
            label_mms(3)

            # ---- divide by counts, write out ----
            out_s = const.tile([BSH, C], f32, name="out_s", tag="out_s")
            nc.vector.tensor_mul(out_s[:], out_ps[:], recb[:])
            nc.sync.dma_start(out_d[:], out_s[:])

    _split_multi_waits(nc)
    return nc


def _prep_host(inputs, support_tensors, support_labels, kernel_w, kernel_b):
    import ml_dtypes

    bf16 = ml_dtypes.bfloat16
    f8 = ml_dtypes.float8_e4m3
    a = np.asarray(inputs, dtype=np.float32)
    S = np.asarray(support_tensors, dtype=np.float32)
    L = np.asarray(support_labels, dtype=np.float32)
    w = np.asarray(kernel_w, dtype=np.float32)
    kb = np.float32(np.asarray(kernel_b, dtype=np.float32))

    aw = ALPHA * 2.0 * np.abs(w)
    sgn = np.sign(w).astype(np.float32)
    s2t = np.ascontiguousarray((S * aw[None, :]).T).astype(bf16)   # [D, N]
    wS = (S @ w).astype(np.float32)                                # [N]
    wa = (a @ w).astype(np.float32)                                # [B]
    a2 = a * aw[None, :]                                           # [B, D]

    sgn_chunks = sgn.reshape(DCH, 128).T                           # [128, DCH]
    # bf16 sliding-window sign tiles (negative slabs): col 31 = -sgn/alpha
    sgnb = np.zeros((128, DCH, 63), dtype=np.float32)
    sgnb[:, :, 31] = -sgn_chunks / ALPHA
    # dense fp8 weight windows, one [128,2,32] slot per fp8 unit
    sgw = np.zeros((128, NF8, 2, 32), dtype=np.float32)
    for j, (prod, p, b) in enumerate(_f8_units()):
        pol = -1.0 if prod == "dve" else 1.0
        for i in range(2):
            sgw[:, j, i, b] = pol * sgn_chunks[:, 2 * p + i] / ALPHA
    sgw = sgw.astype(f8)

    labp = np.zeros((NP, C), dtype=np.float32)
    labp[:N] = L
    labp = labp.reshape(NLAB, 128, C).transpose(1, 0, 2).reshape(128, NLAB * C)
    labsid = np.zeros((128, NLAB * C + 32), dtype=np.float32)
    labsid[:, : NLAB * C] = labp
    labsid[0:32, NLAB * C : NLAB * C + 32] = np.eye(32, dtype=np.float32)
    labsid = labsid.astype(bf16)

    counts = L.sum(axis=0)
    recip = np.where(counts != 0, 1.0 / np.maximum(counts, 1e-30), 0.0)
    recb = np.broadcast_to(recip.astype(np.float32), (BSH, C)).copy()

    shared = {
        "s2t": s2t, "sgnb": sgnb.astype(bf16), "sgw": sgw,
        "labsid": labsid, "recb": recb,
    }
    in_maps = []
    for c in range(NCORES):
        rows = slice(BSH * c, BSH * (c + 1))
        a2t_c = np.ascontiguousarray(
            a2[rows].T.reshape(DCH, 128, BSH))                     # [DCH,128,BSH]
        cc = np.zeros((2, N + BSH), dtype=np.float32)
        cc[0, :N] = 1.0
        cc[1, :N] = wS
        cc[0, N:] = kb - wa[rows]
        cc[1, N:] = 1.0
        in_maps.append(dict(shared, a2t=a2t_c, cc=cc))
    return in_maps


def kernel(**inputs) -> np.ndarray:
    from concourse.bass_utils import run_bass_kernel_spmd

    if "nc" not in _CACHE:
        _CACHE["nc"] = _build_nc()
    nc = _CACHE["nc"]

    in_maps = _prep_host(
        inputs["inputs"], inputs["support_tensors"], inputs["support_labels"],
        inputs["kernel_w"], inputs["kernel_b"],
    )
    res = run_bass_kernel_spmd(nc, in_maps, list(range(NCORES)))
    return np.concatenate([res.results[i]["out"] for i in range(NCORES)], axis=0)


# revision 33
# speedup vs baseline: 1.0500x; 1.0437x over previous
"""Trainium2 Bass kernel for the siamese-kNN classification head.

Reference computation (B=256, N=2000, D=512, C=100):
    scores[b,n] = sigmoid(sum_d w_d * |a[b,d] - S[n,d]| + kb)
    out[b,c]    = (scores @ L)[b,c] / count_c     (0 where count_c == 0)

Strategy
--------
Data-parallel over the batch: core i handles rows 32*i .. 32*i+32, no
collectives.  |x| = 2 relu(x) - x splits the score into a nonlinear slab
relu(A''-S'') (A''= alpha*2|w| (.) a, S''= alpha*2|w| (.) S, bf16) plus an
exact rank-2 f32r correction matmul (kb - w.a_b + (w.S)_n).

The B*N*D slab tensor is the whole cost.  Measured TRN2 engine rates for a
[128,2000] slab chunk:
  DVE tensor_scalar  bf16 out ~760ns   fp8e4 out ~1250ns
  ACT activation     any out  ~1870ns
  PE ingest          bf16 128 elem/cyc; fp8e4 DoubleRow 256 elem/cyc
so slabs are produced in three flavors, tuned so DVE/ACT/PE all finish
together (~82us):
  - 26 rows get one d-chunk-pair as bf16 DVE slabs (PE: 4x [128,500] mm)
  - the other pair of those rows + 6 full rows as fp8 pairs [128,2,2000]
    (DVE x16 pairs, ACT x22), consumed by DoubleRow fp8 matmuls
    ([128,2,250] moving, [128,2,32] sliding sign window) at 2x ingest.
fp8 slabs are alpha=64-scaled so values sit in e4m3's normal range; the
sign windows carry 1/alpha (exact).  End-to-end rel err ~1.5e-2 (fp8
quantization, verified bit-exact against ml_dtypes on host), under the
2e-2 gate.

PE warms up on dummy matmuls during the ~11us DMA cold-start so the
p-state is ramped when the first slab lands.  Tail: sigmoid (ACT) from
PSUM, 16 PE transposes into one PSUM bank, one copy, 16 bf16 label
matmuls, 1/count scale (host-prepared divide-no-nan), DMA out.
"""

import sys

for _p in ("/opt/trn_rl_repo", "/root/.axon_site/_ro/trn_rl_repo"):
    if _p not in sys.path:
        sys.path.append(_p)

import numpy as np

B, N, D, C = 256, 2000, 512, 100
NP = 2048                  # label rows padded to 16 full chunks
NCORES = 8
BSH = B // NCORES          # 32 batch rows per core
DCH = D // 128             # 4 d-chunks
NSEG = 4                   # PSUM free-dim segments
SEG = N // NSEG            # 500
HSEG = SEG // 2            # 250 (DoubleRow moving limit: 2*250 <= 512)
NLAB = NP // 128           # 16 label chunks
ALPHA = 64.0               # fp8 range pre-scale (exact power of 2)

F8_SPLIT = 1               # fp8 matmuls per psc segment (1 -> [128,2,500])
F8C = SEG // F8_SPLIT
KTAIL = 6                  # last units emitted segment-major (early psc stops)

# ---- producer assignment ----
# rows 0..20: mixed -- one chunk-pair bf16 (DVE), the other fp8.
#   even b: fp8 pair = 0; odd b: fp8 pair = 1.
# rows 21..31: both pairs fp8 (21-26 DVE, 27-31 ACT).
_MIX_DVE = frozenset(range(10))   # mixed rows whose fp8 pair runs on DVE
_FULL_DVE = (21, 22, 23, 24, 25, 26)
_FULL_ACT = (27, 28, 29, 30, 31)
N_MIXED = 21

_CACHE = {}


def _f8_units():
    """Canonical order of the fp8 (prod, pair, b) units; index = dense
    weight-window slot (dual-fp8 ldweights needs contiguous aligned
    [128,2,32] weights, so windows are materialized per unit)."""
    order = []
    for p in (0, 1):
        f8_rows = [b for b in range(N_MIXED) if (b % 2 == 0) == (p == 0)]
        for b in [x for x in f8_rows if x in _MIX_DVE] + list(_FULL_DVE):
            order.append(("dve", p, b))
        for b in [x for x in f8_rows if x not in _MIX_DVE] + list(_FULL_ACT):
            order.append(("act", p, b))
    return order


NF8 = len(_f8_units())


def _plan():
    """Static schedule: producer instruction order + PE consumption order.

    Items: dict(kind='bfs', ch, b) one bf16 chunk-slab, or
           dict(kind='f8p', pair, b, prod) one fp8 chunk-pair.
    Emission order of PE matmuls = estimated completion order, so the
    in-order PE never waits on a later-finishing producer while an
    earlier slab sits ready.
    """
    # trace-calibrated: first-DMA land / chunk spacing / engine first-instr
    CH_LAND = [12400.0, 14000.0, 15500.0, 17000.0]
    AB_LAND = 12400.0
    DVE_T0, ACT_T0 = 12400.0, 14100.0
    T_BF, T_F8H, T_AH = 755.0, 1140.0, 1860.0

    def interleave(la, lb):
        out, ia, ib = [], 0, 0
        while ia < len(la) or ib < len(lb):
            if ib >= len(lb) or (ia < len(la) and ia * len(lb) <= ib * len(la)):
                out.append(la[ia]); ia += 1
            else:
                out.append(lb[ib]); ib += 1
        return out

    items, dve_prog, act_prog = [], [], []
    for p in (0, 1):
        ch0, ch1 = 2 * p, 2 * p + 1
        bf_rows = [b for b in range(N_MIXED) if (b % 2 == 0) == (p == 1)]
        f8_rows = [b for b in range(N_MIXED) if (b % 2 == 0) == (p == 0)]
        f8_dve = [b for b in f8_rows if b in _MIX_DVE] + list(_FULL_DVE)
        f8_act = [b for b in f8_rows if b not in _MIX_DVE] + list(_FULL_ACT)

        s0 = [dict(kind="bfs", ch=ch0, b=b) for b in bf_rows]
        s1 = [dict(kind="bfs", ch=ch1, b=b) for b in bf_rows]
        fv = [dict(kind="f8p", pair=p, b=b, prod="dve") for b in f8_dve]
        fa = [dict(kind="f8p", pair=p, b=b, prod="act") for b in f8_act]
        items += s0 + s1 + fv + fa

        dve_prog += [("bfs", u) for u in s0]
        for entry in interleave([("bfs", u) for u in s1],
                                [("f8p", u) for u in fv]):
            if entry[0] == "bfs":
                dve_prog.append(entry)
            else:
                dve_prog.append(("f8h", entry[1], 0))
                dve_prog.append(("f8h", entry[1], 1))
        for u in fa:
            act_prog.append(("f8h", u, 0))
            act_prog.append(("f8h", u, 1))

    def sim(prog, t_bf, t_half, clock):
        for entry in prog:
            if entry[0] == "bfs":
                u = entry[1]
                clock = max(clock, CH_LAND[u["ch"]], AB_LAND) + t_bf
                u["done"] = clock
            else:
                _, u, h = entry
                clock = max(clock, CH_LAND[2 * u["pair"] + h], AB_LAND) + t_half
                if h == 1:
                    u["done"] = clock

    sim(dve_prog, T_BF, T_F8H, DVE_T0)
    sim(act_prog, 0.0, T_AH, ACT_T0)
    widx = {u: j for j, u in enumerate(_f8_units())}
    for u in items:
        if u["kind"] == "f8p":
            u["widx"] = widx[(u["prod"], u["pair"], u["b"])]
    pe_order = sorted(items, key=lambda u: u["done"])
    assert pe_order[0]["kind"] == "bfs" and pe_order[0]["ch"] == 0
    return dve_prog, act_prog, pe_order


def _split_multi_waits(nc):
    """TRN2 TPB instructions encode at most ONE semaphore wait; split extras
    into single-wait NOPs directly before the instruction (same engine)."""
    from concourse import mybir

    for fn in nc.m.functions:
        for bb in fn.blocks:
            out = []
            for inst in bb.instructions:
                si = inst.sync_info
                if si is not None and si.on_wait and len(si.on_wait) > 1:
                    waits = list(si.on_wait)
                    for j, w in enumerate(waits[:-1]):
                        out.append(mybir.InstNoOp(
                            name=f"{inst.name}-sw{j}", engine=inst.engine,
                            sync_info=mybir.SyncInfo(on_wait=[w], on_update=[]),
                            ins=[], outs=[]))
                    inst.sync_info = mybir.SyncInfo(
                        on_wait=[waits[-1]], on_update=list(si.on_update))
                out.append(inst)
            bb.instructions = out


def _build_nc():
    import concourse.bass as bass
    import concourse.tile as tile
    from concourse import mybir

    f32 = mybir.dt.float32
    f32r = mybir.dt.float32r
    bf16 = mybir.dt.bfloat16
    f8 = mybir.dt.float8e4
    nc = bass.Bass()

    s2t_d = nc.declare_dram_parameter("s2t", [D, N], bf16, isOutput=False)
    a2t_d = nc.declare_dram_parameter("a2t", [DCH, 128, BSH], f32, isOutput=False)
    sgnb_d = nc.declare_dram_parameter("sgnb", [128, DCH, 63], bf16, isOutput=False)
    sgw_d = nc.declare_dram_parameter("sgw", [128, NF8, 2, 32], f8, isOutput=False)
    cc_d = nc.declare_dram_parameter("cc", [2, N + BSH], f32r, isOutput=False)
    labsid_d = nc.declare_dram_parameter("labsid", [128, NLAB * C + 32],
                                         bf16, isOutput=False)
    recb_d = nc.declare_dram_parameter("recb", [BSH, C], f32, isOutput=False)
    out_d = nc.declare_dram_parameter("out", [BSH, C], f32, isOutput=True)

    dve_prog, act_prog, pe_order = _plan()
    sub = None  # set after mybir import below

    with tile.TileContext(nc) as tc:
        with (
            tc.tile_pool(name="const", bufs=1) as const,
            tc.tile_pool(name="dslab", bufs=6) as dpool,
            tc.tile_pool(name="vpair", bufs=3) as vpool,
            tc.tile_pool(name="apair", bufs=3) as apool,
            tc.tile_pool(name="bank", bufs=8, space="PSUM") as bankp,
        ):
            # ---- DMAs: s2t chunks (big, latency-critical) serial on the
            # Sync queue -- first chunk split in halves so the first slab
            # starts ~1us earlier.  Small constants go on the (otherwise
            # idle) GpSimd queue in parallel; big late-needed tensors (sgw,
            # labsid) follow the chunks on Sync to keep the first-chunk
            # window free of bandwidth contention.
            s2t0 = const.tile([128, N], bf16, name="s2t0", tag="s2t0")
            nc.sync.dma_start(s2t0[:, 0 : N // 2], s2t_d[0:128, 0 : N // 2])
            nc.sync.dma_start(s2t0[:, N // 2 : N], s2t_d[0:128, N // 2 : N])
            s2t123 = const.tile([128, 3, N], bf16, name="s2t123", tag="s2t123")
            nc.sync.dma_start(s2t123[:, 0, :], s2t_d[128:256, :])
            nc.sync.dma_start(s2t123[:, 1, :], s2t_d[256:384, :])
            nc.sync.dma_start(s2t123[:, 2, :], s2t_d[384:512, :])
            sgw = const.tile([128, NF8, 2, 32], f8, name="sgw", tag="sgw")
            nc.sync.dma_start(sgw[:], sgw_d[:])
            labsid = const.tile([128, NLAB * C + 32], bf16,
                               name="labsid", tag="labsid")
            nc.sync.dma_start(labsid[:], labsid_d[:])
            a2t = const.tile([128, DCH * BSH], f32, name="a2t", tag="a2t")
            nc.gpsimd.dma_start(
                a2t[:].rearrange("p (c b) -> p c b", c=DCH),
                a2t_d[:].rearrange("c p b -> p c b"),
            )
            sgnb = const.tile([128, DCH, 63], bf16, name="sgnb", tag="sgnb")
            nc.gpsimd.dma_start(sgnb[:], sgnb_d[:])
            cc = const.tile([2, N + BSH], f32r, name="cc", tag="cc")
            nc.gpsimd.dma_start(cc[:], cc_d[:])
            recb = const.tile([BSH, C], f32, name="recb", tag="recb")
            nc.gpsimd.dma_start(recb[:], recb_d[:])

            s2t = [s2t0] + [s2t123[:, k, :] for k in range(3)]
            ident = labsid[0:32, NLAB * C : NLAB * C + 32]

            sub_op = mybir.AluOpType.subtract
            min_op = mybir.AluOpType.min
            relu = mybir.ActivationFunctionType.Relu

            psc = [
                bankp.tile([BSH, SEG], f32, name=f"psc{s}", tag="bank")
                for s in range(NSEG)
            ]

            # ---- PE p-state warmup while DMAs run
            dummy_sb = const.tile([128, 512], bf16, name="dummy", tag="dummy")
            nc.vector.memset(dummy_sb[:], 0.0)
            dummy_ps = bankp.tile([2, 512], f32, name="dummy_ps", tag="bank")
            for _ in range(15):
                nc.tensor.matmul(
                    dummy_ps[:], dummy_sb[:, 0:2], dummy_sb[:],
                    start=True, stop=True, skip_group_check=True,
                )

            # ---- producers (DVE / ACT program order from the plan)
            n_bfs_seen = 0
            for entry in dve_prog:
                if entry[0] == "bfs":
                    u = entry[1]
                    slab = dpool.tile([128, N], bf16, name="dslab", tag="dslab")
                    col = u["ch"] * BSH + u["b"]
                    if n_bfs_seen < 2:
                        # first slabs chase the split s2t0 halves
                        for lo, hi in ((0, N // 2), (N // 2, N)):
                            nc.vector.tensor_scalar(
                                slab[:, lo:hi], s2t[u["ch"]][:, lo:hi],
                                a2t[:, col : col + 1], 0.0, sub_op, min_op,
                            )
                    else:
                        nc.vector.tensor_scalar(
                            slab[:], s2t[u["ch"]], a2t[:, col : col + 1],
                            0.0, sub_op, min_op,
                        )
                    n_bfs_seen += 1
                    u["ap"] = slab
                else:
                    _, u, h = entry
                    if h == 0:
                        u["ap"] = vpool.tile([128, NSEG * F8_SPLIT, 2, F8C], f8,
                                             name="vpair", tag="vpair")
                    ch = 2 * u["pair"] + h
                    nc.vector.tensor_scalar(
                        u["ap"][:, :, h, :], s2t[ch],
                        a2t[:, ch * BSH + u["b"] : ch * BSH + u["b"] + 1],
                        0.0, sub_op, min_op,
                    )
            for entry in act_prog:
                _, u, h = entry
                if h == 0:
                    u["ap"] = apool.tile([128, NSEG * F8_SPLIT, 2, F8C], f8,
                                         name="apair", tag="apair")
                ch = 2 * u["pair"] + h
                nc.scalar.activation(
                    u["ap"][:, :, h, :], s2t[ch], relu,
                    bias=a2t[:, ch * BSH + u["b"] : ch * BSH + u["b"] + 1],
                    scale=-1.0,
                )

            # ---- PE stream in estimated completion order
            def unit_seg_mms(u, s, first, stop):
                b = u["b"]
                if u["kind"] == "bfs":
                    nc.tensor.matmul(
                        psc[s][:], sgnb[:, u["ch"], 31 - b : 63 - b],
                        u["ap"][:, SEG * s : SEG * (s + 1)],
                        start=first, stop=stop,
                        skip_group_check=not first,
                    )
                else:
                    for j in range(F8_SPLIT):
                        nc.tensor.matmul(
                            psc[s][:, F8C * j : F8C * (j + 1)],
                            sgw[:, u["widx"], :, :],
                            u["ap"][:, F8_SPLIT * s + j, :, :],
                            start=False, stop=(stop and j == F8_SPLIT - 1),
                            perf_mode=mybir.MatmulPerfMode.DoubleRow,
                            skip_group_check=True,
                        )

            last_idx = len(pe_order) - 1
            for idx, u in enumerate(pe_order):
                for s in range(NSEG):
                    unit_seg_mms(u, s, first=(idx == 0), stop=(idx == last_idx))
                if idx == 6:
                    # exact rank-2 correction: kb - w.a_b + (w.S)_n
                    for s in range(NSEG):
                        nc.tensor.matmul(
                            psc[s][:], cc[:, N : N + BSH],
                            cc[:, SEG * s : SEG * (s + 1)],
                            start=False, stop=False, skip_group_check=True,
                        )

            # ---- sigmoid (PSUM -> SBUF, bf16) ----
            ssig = const.tile([BSH, N], bf16, name="ssig", tag="ssig")
            for s in range(NSEG):
                nc.scalar.activation(
                    ssig[:, SEG * s : SEG * (s + 1)], psc[s][:],
                    mybir.ActivationFunctionType.Sigmoid,
                )

            # ---- transposes into ONE PSUM bank, quartet-pipelined copy +
            # label matmuls (copy chunk j frees transposes 4j..4j+3)
            tpall = bankp.tile([128, NLAB * BSH], bf16, name="tpall", tag="bank")
            sct = const.tile([128, NLAB * BSH], bf16, name="sct", tag="sct")
            out_ps = bankp.tile([BSH, C], f32, name="out_ps", tag="bank")
            for k in range(NLAB):
                pk = min(128, N - 128 * k)
                nc.tensor.transpose(
                    tpall[:pk, BSH * k : BSH * k + BSH],
                    ssig[:, 128 * k : 128 * k + pk], ident,
                )
            nc.vector.tensor_copy(sct[:], tpall[:])
            for k in range(NLAB):
                pk = min(128, N - 128 * k)
                nc.tensor.matmul(
                    out_ps[:], sct[:pk, BSH * k : BSH * k + BSH],
                    labsid[:pk, C * k : C * (k + 1)],
                    start=(k == 0), stop=(k == NLAB - 1),
                )

            # ---- divide by counts, write out ----
            out_s = const.tile([BSH, C], f32, name="out_s", tag="out_s")
            nc.vector.tensor_mul(out_s[:], out_ps[:], recb[:])
            nc.sync.dma_start(out_d[:], out_s[:])

    _split_multi_waits(nc)
    return nc


def _prep_host(inputs, support_tensors, support_labels, kernel_w, kernel_b):
    import ml_dtypes

    bf16 = ml_dtypes.bfloat16
    f8 = ml_dtypes.float8_e4m3
    a = np.asarray(inputs, dtype=np.float32)
    S = np.asarray(support_tensors, dtype=np.float32)
    L = np.asarray(support_labels, dtype=np.float32)
    w = np.asarray(kernel_w, dtype=np.float32)
    kb = np.float32(np.asarray(kernel_b, dtype=np.float32))

    aw = ALPHA * 2.0 * np.abs(w)
    sgn = np.sign(w).astype(np.float32)
    s2t = np.ascontiguousarray((S * aw[None, :]).T).astype(bf16)   # [D, N]
    wS = (S @ w).astype(np.float32)                                # [N]
    wa = (a @ w).astype(np.float32)                                # [B]
    a2 = a * aw[None, :]                                           # [B, D]

    sgn_chunks = sgn.reshape(DCH, 128).T                           # [128, DCH]
    # bf16 sliding-window sign tiles (negative slabs): col 31 = -sgn/alpha
    sgnb = np.zeros((128, DCH, 63), dtype=np.float32)
    sgnb[:, :, 31] = -sgn_chunks / ALPHA
    # dense fp8 weight windows, one [128,2,32] slot per fp8 unit
    sgw = np.zeros((128, NF8, 2, 32), dtype=np.float32)
    for j, (prod, p, b) in enumerate(_f8_units()):
        pol = -1.0 if prod == "dve" else 1.0
        for i in range(2):
            sgw[:, j, i, b] = pol * sgn_chunks[:, 2 * p + i] / ALPHA
    sgw = sgw.astype(f8)

    labp = np.zeros((NP, C), dtype=np.float32)
    labp[:N] = L
    labp = labp.reshape(NLAB, 128, C).transpose(1, 0, 2).reshape(128, NLAB * C)
    labsid = np.zeros((128, NLAB * C + 32), dtype=np.float32)
    labsid[:, : NLAB * C] = labp
    labsid[0:32, NLAB * C : NLAB * C + 32] = np.eye(32, dtype=np.float32)
    labsid = labsid.astype(bf16)

    counts = L.sum(axis=0)
    recip = np.where(counts != 0, 1.0 / np.maximum(counts, 1e-30), 0.0)
    recb = np.broadcast_to(recip.astype(np.float32), (BSH, C)).copy()

    shared = {
        "s2t": s2t, "sgnb": sgnb.astype(bf16), "sgw": sgw,
        "labsid": labsid, "recb": recb,
    }
    in_maps = []
    for c in range(NCORES):
        rows = slice(BSH * c, BSH * (c + 1))
        a2t_c = np.ascontiguousarray(
            a2[rows].T.reshape(DCH, 128, BSH))                     # [DCH,128,BSH]
        cc = np.zeros((2, N + BSH), dtype=np.float32)
        cc[0, :N] = 1.0
        cc[1, :N] = wS
        cc[0, N:] = kb - wa[rows]
        cc[1, N:] = 1.0
        in_maps.append(dict(shared, a2t=a2t_c, cc=cc))
    return in_maps


def kernel(**inputs) -> np.ndarray:
    from concourse.bass_utils import run_bass_kernel_spmd

    if "nc" not in _CACHE:
        _CACHE["nc"] = _build_nc()
    nc = _CACHE["nc"]

    in_maps = _prep_host(
        inputs["inputs"], inputs["support_tensors"], inputs["support_labels"],
        inputs["kernel_w"], inputs["kernel_b"],
    )
    res = run_bass_kernel_spmd(nc, in_maps, list(range(NCORES)))
    return np.concatenate([res.results[i]["out"] for i in range(NCORES)], axis=0)


# revision 36
# speedup vs baseline: 1.0501x; 1.0001x over previous
"""Trainium2 Bass kernel for the siamese-kNN classification head.

Reference computation (B=256, N=2000, D=512, C=100):
    scores[b,n] = sigmoid(sum_d w_d * |a[b,d] - S[n,d]| + kb)
    out[b,c]    = (scores @ L)[b,c] / count_c     (0 where count_c == 0)

Strategy
--------
Data-parallel over the batch: core i handles rows 32*i .. 32*i+32, no
collectives.  |x| = 2 relu(x) - x splits the score into a nonlinear slab
relu(A''-S'') (A''= alpha*2|w| (.) a, S''= alpha*2|w| (.) S, bf16) plus an
exact rank-2 f32r correction matmul (kb - w.a_b + (w.S)_n).

The B*N*D slab tensor is the whole cost.  Measured TRN2 engine rates for a
[128,2000] slab chunk:
  DVE tensor_scalar  bf16 out ~755ns   fp8e4 out ~1140ns
  ACT activation     any out  ~1860ns
  PE ingest          bf16 128 elem/cyc; fp8e4 DoubleRow 256 elem/cyc
so slabs are produced in three flavors, tuned so DVE/ACT/PE all finish
their ~79-80us of stream work together:
  - 21 mixed rows get one d-chunk-pair as bf16 DVE slabs (4x [128,500] mm)
  - the other pair of those rows + 11 full rows as alpha=64-scaled fp8
    pairs (DVE x22, ACT x21 units), consumed by dual-row fp8 matmuls
    ([128,2,500] moving k-tile-concatenated, dense aligned [128,2,32]
    weight windows -- walrus dual-fp8 ldweights rejects strided/odd-offset
    weights) at 2x ingest.  End-to-end rel err 1.53e-2 (fp8 quantization,
    bit-exact vs ml_dtypes emulation on host), under the 2e-2 gate.

Producer/PE instruction order comes from a trace-calibrated completion-time
simulation (_plan) so the in-order PE almost never stalls (measured <3us
idle).  PE warms up on dummy matmuls during the ~12us NEFF+DMA cold-start;
the first s2t chunk is DMA-split so the first slab starts earlier; small
constants ride the idle GpSimd DMA queue.  Tail: per-segment sigmoid (ACT)
from PSUM overlapped with the last matmuls, 16 PE transposes into one PSUM
bank, one copy, 16 bf16 label matmuls, 1/count scale (host-prepared
divide-no-nan), DMA out.  133.5us baseline -> 101.5us.
"""

import sys

for _p in ("/opt/trn_rl_repo", "/root/.axon_site/_ro/trn_rl_repo"):
    if _p not in sys.path:
        sys.path.append(_p)

import numpy as np

B, N, D, C = 256, 2000, 512, 100
NP = 2048                  # label rows padded to 16 full chunks
NCORES = 8
BSH = B // NCORES          # 32 batch rows per core
DCH = D // 128             # 4 d-chunks
NSEG = 4                   # PSUM free-dim segments
SEG = N // NSEG            # 500
HSEG = SEG // 2            # 250 (DoubleRow moving limit: 2*250 <= 512)
NLAB = NP // 128           # 16 label chunks
ALPHA = 64.0               # fp8 range pre-scale (exact power of 2)

F8_SPLIT = 1               # fp8 matmuls per psc segment (1 -> [128,2,500])
F8C = SEG // F8_SPLIT
KTAIL = 6                  # last units emitted segment-major (early psc stops)

# ---- producer assignment ----
# rows 0..20: mixed -- one chunk-pair bf16 (DVE), the other fp8.
#   even b: fp8 pair = 0; odd b: fp8 pair = 1.
# rows 21..31: both pairs fp8 (21-26 DVE, 27-31 ACT).
_MIX_DVE = frozenset(range(10))   # mixed rows whose fp8 pair runs on DVE
_FULL_DVE = (21, 22, 23, 24, 25, 26)
_FULL_ACT = (27, 28, 29, 30, 31)
N_MIXED = 21

_CACHE = {}


def _f8_units():
    """Canonical order of the fp8 (prod, pair, b) units; index = dense
    weight-window slot (dual-fp8 ldweights needs contiguous aligned
    [128,2,32] weights, so windows are materialized per unit)."""
    order = []
    for p in (0, 1):
        f8_rows = [b for b in range(N_MIXED) if (b % 2 == 0) == (p == 0)]
        for b in [x for x in f8_rows if x in _MIX_DVE] + list(_FULL_DVE):
            order.append(("dve", p, b))
        for b in [x for x in f8_rows if x not in _MIX_DVE] + list(_FULL_ACT):
            order.append(("act", p, b))
    return order


NF8 = len(_f8_units())


def _plan():
    """Static schedule: producer instruction order + PE consumption order.

    Items: dict(kind='bfs', ch, b) one bf16 chunk-slab, or
           dict(kind='f8p', pair, b, prod) one fp8 chunk-pair.
    Emission order of PE matmuls = estimated completion order, so the
    in-order PE never waits on a later-finishing producer while an
    earlier slab sits ready.
    """
    # trace-calibrated: first-DMA land / chunk spacing / engine first-instr
    CH_LAND = [12400.0, 14000.0, 15500.0, 17000.0]
    AB_LAND = 12400.0
    DVE_T0, ACT_T0 = 12400.0, 14100.0
    T_BF, T_F8H, T_AH = 755.0, 1140.0, 1860.0

    def interleave(la, lb):
        out, ia, ib = [], 0, 0
        while ia < len(la) or ib < len(lb):
            if ib >= len(lb) or (ia < len(la) and ia * len(lb) <= ib * len(la)):
                out.append(la[ia]); ia += 1
            else:
                out.append(lb[ib]); ib += 1
        return out

    items, dve_prog, act_prog = [], [], []
    for p in (0, 1):
        ch0, ch1 = 2 * p, 2 * p + 1
        bf_rows = [b for b in range(N_MIXED) if (b % 2 == 0) == (p == 1)]
        f8_rows = [b for b in range(N_MIXED) if (b % 2 == 0) == (p == 0)]
        f8_dve = [b for b in f8_rows if b in _MIX_DVE] + list(_FULL_DVE)
        f8_act = [b for b in f8_rows if b not in _MIX_DVE] + list(_FULL_ACT)

        s0 = [dict(kind="bfs", ch=ch0, b=b) for b in bf_rows]
        s1 = [dict(kind="bfs", ch=ch1, b=b) for b in bf_rows]
        fv = [dict(kind="f8p", pair=p, b=b, prod="dve") for b in f8_dve]
        fa = [dict(kind="f8p", pair=p, b=b, prod="act") for b in f8_act]
        items += s0 + s1 + fv + fa

        dve_prog += [("bfs", u) for u in s0]
        for entry in interleave([("bfs", u) for u in s1],
                                [("f8p", u) for u in fv]):
            if entry[0] == "bfs":
                dve_prog.append(entry)
            else:
                dve_prog.append(("f8h", entry[1], 0))
                dve_prog.append(("f8h", entry[1], 1))
        for u in fa:
            act_prog.append(("f8h", u, 0))
            act_prog.append(("f8h", u, 1))

    def sim(prog, t_bf, t_half, clock):
        for entry in prog:
            if entry[0] == "bfs":
                u = entry[1]
                clock = max(clock, CH_LAND[u["ch"]], AB_LAND) + t_bf
                u["done"] = clock
            else:
                _, u, h = entry
                clock = max(clock, CH_LAND[2 * u["pair"] + h], AB_LAND) + t_half
                if h == 1:
                    u["done"] = clock

    sim(dve_prog, T_BF, T_F8H, DVE_T0)
    sim(act_prog, 0.0, T_AH, ACT_T0)
    widx = {u: j for j, u in enumerate(_f8_units())}
    for u in items:
        if u["kind"] == "f8p":
            u["widx"] = widx[(u["prod"], u["pair"], u["b"])]
    pe_order = sorted(items, key=lambda u: u["done"])
    assert pe_order[0]["kind"] == "bfs" and pe_order[0]["ch"] == 0
    return dve_prog, act_prog, pe_order


def _split_multi_waits(nc):
    """TRN2 TPB instructions encode at most ONE semaphore wait; split extras
    into single-wait NOPs directly before the instruction (same engine)."""
    from concourse import mybir

    for fn in nc.m.functions:
        for bb in fn.blocks:
            out = []
            for inst in bb.instructions:
                si = inst.sync_info
                if si is not None and si.on_wait and len(si.on_wait) > 1:
                    waits = list(si.on_wait)
                    for j, w in enumerate(waits[:-1]):
                        out.append(mybir.InstNoOp(
                            name=f"{inst.name}-sw{j}", engine=inst.engine,
                            sync_info=mybir.SyncInfo(on_wait=[w], on_update=[]),
                            ins=[], outs=[]))
                    inst.sync_info = mybir.SyncInfo(
                        on_wait=[waits[-1]], on_update=list(si.on_update))
                out.append(inst)
            bb.instructions = out


def _build_nc():
    import concourse.bass as bass
    import concourse.tile as tile
    from concourse import mybir

    f32 = mybir.dt.float32
    f32r = mybir.dt.float32r
    bf16 = mybir.dt.bfloat16
    f8 = mybir.dt.float8e4
    nc = bass.Bass()

    s2t_d = nc.declare_dram_parameter("s2t", [D, N], bf16, isOutput=False)
    a2t_d = nc.declare_dram_parameter("a2t", [DCH, 128, BSH], f32, isOutput=False)
    sgnb_d = nc.declare_dram_parameter("sgnb", [128, DCH, 63], bf16, isOutput=False)
    sgw_d = nc.declare_dram_parameter("sgw", [128, NF8, 2, 32], f8, isOutput=False)
    cc_d = nc.declare_dram_parameter("cc", [2, N + BSH], f32r, isOutput=False)
    labsid_d = nc.declare_dram_parameter("labsid", [128, NLAB * C + 32],
                                         bf16, isOutput=False)
    recb_d = nc.declare_dram_parameter("recb", [BSH, C], f32, isOutput=False)
    out_d = nc.declare_dram_parameter("out", [BSH, C], f32, isOutput=True)

    dve_prog, act_prog, pe_order = _plan()
    sub = None  # set after mybir import below

    with tile.TileContext(nc) as tc:
        with (
            tc.tile_pool(name="const", bufs=1) as const,
            tc.tile_pool(name="dslab", bufs=6) as dpool,
            tc.tile_pool(name="vpair", bufs=4) as vpool,
            tc.tile_pool(name="apair", bufs=4) as apool,
            tc.tile_pool(name="bank", bufs=8, space="PSUM") as bankp,
        ):
            # ---- DMAs: s2t chunks (big, latency-critical) serial on the
            # Sync queue -- first chunk split in halves so the first slab
            # starts ~1us earlier.  Small constants go on the (otherwise
            # idle) GpSimd queue in parallel; big late-needed tensors (sgw,
            # labsid) follow the chunks on Sync to keep the first-chunk
            # window free of bandwidth contention.
            s2t0 = const.tile([128, N], bf16, name="s2t0", tag="s2t0")
            nc.sync.dma_start(s2t0[:, 0 : N // 2], s2t_d[0:128, 0 : N // 2])
            nc.sync.dma_start(s2t0[:, N // 2 : N], s2t_d[0:128, N // 2 : N])
            s2t123 = const.tile([128, 3, N], bf16, name="s2t123", tag="s2t123")
            nc.sync.dma_start(s2t123[:, 0, :], s2t_d[128:256, :])
            nc.sync.dma_start(s2t123[:, 1, :], s2t_d[256:384, :])
            nc.sync.dma_start(s2t123[:, 2, :], s2t_d[384:512, :])
            sgw = const.tile([128, NF8, 2, 32], f8, name="sgw", tag="sgw")
            nc.sync.dma_start(sgw[:], sgw_d[:])
            labsid = const.tile([128, NLAB * C + 32], bf16,
                               name="labsid", tag="labsid")
            nc.sync.dma_start(labsid[:], labsid_d[:])
            a2t = const.tile([128, DCH * BSH], f32, name="a2t", tag="a2t")
            nc.gpsimd.dma_start(
                a2t[:].rearrange("p (c b) -> p c b", c=DCH),
                a2t_d[:].rearrange("c p b -> p c b"),
            )
            sgnb = const.tile([128, DCH, 63], bf16, name="sgnb", tag="sgnb")
            nc.gpsimd.dma_start(sgnb[:], sgnb_d[:])
            cc = const.tile([2, N + BSH], f32r, name="cc", tag="cc")
            nc.gpsimd.dma_start(cc[:], cc_d[:])
            recb = const.tile([BSH, C], f32, name="recb", tag="recb")
            nc.gpsimd.dma_start(recb[:], recb_d[:])

            s2t = [s2t0] + [s2t123[:, k, :] for k in range(3)]
            ident = labsid[0:32, NLAB * C : NLAB * C + 32]

            sub_op = mybir.AluOpType.subtract
            min_op = mybir.AluOpType.min
            relu = mybir.ActivationFunctionType.Relu

            psc = [
                bankp.tile([BSH, SEG], f32, name=f"psc{s}", tag="bank")
                for s in range(NSEG)
            ]

            # ---- PE p-state warmup while DMAs run
            dummy_sb = const.tile([128, 512], bf16, name="dummy", tag="dummy")
            nc.vector.memset(dummy_sb[:], 0.0)
            dummy_ps = bankp.tile([2, 512], f32, name="dummy_ps", tag="bank")
            for _ in range(15):
                nc.tensor.matmul(
                    dummy_ps[:], dummy_sb[:, 0:2], dummy_sb[:],
                    start=True, stop=True, skip_group_check=True,
                )

            # ---- producers (DVE / ACT program order from the plan)
            n_bfs_seen = 0
            for entry in dve_prog:
                if entry[0] == "bfs":
                    u = entry[1]
                    slab = dpool.tile([128, N], bf16, name="dslab", tag="dslab")
                    col = u["ch"] * BSH + u["b"]
                    if n_bfs_seen < 2:
                        # first slabs chase the split s2t0 halves
                        for lo, hi in ((0, N // 2), (N // 2, N)):
                            nc.vector.tensor_scalar(
                                slab[:, lo:hi], s2t[u["ch"]][:, lo:hi],
                                a2t[:, col : col + 1], 0.0, sub_op, min_op,
                            )
                    else:
                        nc.vector.tensor_scalar(
                            slab[:], s2t[u["ch"]], a2t[:, col : col + 1],
                            0.0, sub_op, min_op,
                        )
                    n_bfs_seen += 1
                    u["ap"] = slab
                else:
                    _, u, h = entry
                    if h == 0:
                        u["ap"] = vpool.tile([128, NSEG * F8_SPLIT, 2, F8C], f8,
                                             name="vpair", tag="vpair")
                    ch = 2 * u["pair"] + h
                    nc.vector.tensor_scalar(
                        u["ap"][:, :, h, :], s2t[ch],
                        a2t[:, ch * BSH + u["b"] : ch * BSH + u["b"] + 1],
                        0.0, sub_op, min_op,
                    )
            for entry in act_prog:
                _, u, h = entry
                if h == 0:
                    u["ap"] = apool.tile([128, NSEG * F8_SPLIT, 2, F8C], f8,
                                         name="apair", tag="apair")
                ch = 2 * u["pair"] + h
                nc.scalar.activation(
                    u["ap"][:, :, h, :], s2t[ch], relu,
                    bias=a2t[:, ch * BSH + u["b"] : ch * BSH + u["b"] + 1],
                    scale=-1.0,
                )

            # ---- PE stream in estimated completion order
            def unit_seg_mms(u, s, first, stop):
                b = u["b"]
                if u["kind"] == "bfs":
                    nc.tensor.matmul(
                        psc[s][:], sgnb[:, u["ch"], 31 - b : 63 - b],
                        u["ap"][:, SEG * s : SEG * (s + 1)],
                        start=first, stop=stop,
                        skip_group_check=not first,
                    )
                else:
                    for j in range(F8_SPLIT):
                        nc.tensor.matmul(
                            psc[s][:, F8C * j : F8C * (j + 1)],
                            sgw[:, u["widx"], :, :],
                            u["ap"][:, F8_SPLIT * s + j, :, :],
                            start=False, stop=(stop and j == F8_SPLIT - 1),
                            perf_mode=mybir.MatmulPerfMode.DoubleRow,
                            skip_group_check=True,
                        )

            last_idx = len(pe_order) - 1
            for idx, u in enumerate(pe_order):
                for s in range(NSEG):
                    unit_seg_mms(u, s, first=(idx == 0), stop=(idx == last_idx))
                if idx == 6:
                    # exact rank-2 correction: kb - w.a_b + (w.S)_n
                    for s in range(NSEG):
                        nc.tensor.matmul(
                            psc[s][:], cc[:, N : N + BSH],
                            cc[:, SEG * s : SEG * (s + 1)],
                            start=False, stop=False, skip_group_check=True,
                        )

            # ---- sigmoid (PSUM -> SBUF, bf16) ----
            ssig = const.tile([BSH, N], bf16, name="ssig", tag="ssig")
            for s in range(NSEG):
                nc.scalar.activation(
                    ssig[:, SEG * s : SEG * (s + 1)], psc[s][:],
                    mybir.ActivationFunctionType.Sigmoid,
                )

            # ---- transposes into ONE PSUM bank, quartet-pipelined copy +
            # label matmuls (copy chunk j frees transposes 4j..4j+3)
            tpall = bankp.tile([128, NLAB * BSH], bf16, name="tpall", tag="bank")
            sct = const.tile([128, NLAB * BSH], bf16, name="sct", tag="sct")
            out_ps = bankp.tile([BSH, C], f32, name="out_ps", tag="bank")
            for k in range(NLAB):
                pk = min(128, N - 128 * k)
                nc.tensor.transpose(
                    tpall[:pk, BSH * k : BSH * k + BSH],
                    ssig[:, 128 * k : 128 * k + pk], ident,
                )
            # 2-way copy so the first 8 label matmuls overlap the 2nd copy
            nc.vector.tensor_copy(sct[:, : BSH * 8], tpall[:, : BSH * 8])
            nc.vector.tensor_copy(sct[:, BSH * 8 :], tpall[:, BSH * 8 :])
            for k in range(NLAB):
                pk = min(128, N - 128 * k)
                nc.tensor.matmul(
                    out_ps[:], sct[:pk, BSH * k : BSH * k + BSH],
                    labsid[:pk, C * k : C * (k + 1)],
                    start=(k == 0), stop=(k == NLAB - 1),
                )

            # ---- divide by counts, write out ----
            out_s = const.tile([BSH, C], f32, name="out_s", tag="out_s")
            nc.vector.tensor_mul(out_s[:], out_ps[:], recb[:])
            nc.sync.dma_start(out_d[:], out_s[:])

    _split_multi_waits(nc)
    return nc


def _prep_host(inputs, support_tensors, support_labels, kernel_w, kernel_b):
    import ml_dtypes

    bf16 = ml_dtypes.bfloat16
    f8 = ml_dtypes.float8_e4m3
    a = np.asarray(inputs, dtype=np.float32)
    S = np.asarray(support_tensors, dtype=np.float32)
    L = np.asarray(support_labels, dtype=np.float32)
    w = np.asarray(kernel_w, dtype=np.float32)
    kb = np.float32(np.asarray(kernel_b, dtype=np.float32))

    aw = ALPHA * 2.0 * np.abs(w)
    sgn = np.sign(w).astype(np.float32)
    s2t = np.ascontiguousarray((S * aw[None, :]).T).astype(bf16)   # [D, N]
    wS = (S @ w).astype(np.float32)                                # [N]
    wa = (a @ w).astype(np.float32)                                # [B]
    a2 = a * aw[None, :]                                           # [B, D]

    sgn_chunks = sgn.reshape(DCH, 128).T                           # [128, DCH]
    # bf16 sliding-window sign tiles (negative slabs): col 31 = -sgn/alpha
    sgnb = np.zeros((128, DCH, 63), dtype=np.float32)
    sgnb[:, :, 31] = -sgn_chunks / ALPHA
    # dense fp8 weight windows, one [128,2,32] slot per fp8 unit
    sgw = np.zeros((128, NF8, 2, 32), dtype=np.float32)
    for j, (prod, p, b) in enumerate(_f8_units()):
        pol = -1.0 if prod == "dve" else 1.0
        for i in range(2):
            sgw[:, j, i, b] = pol * sgn_chunks[:, 2 * p + i] / ALPHA
    sgw = sgw.astype(f8)

    labp = np.zeros((NP, C), dtype=np.float32)
    labp[:N] = L
    labp = labp.reshape(NLAB, 128, C).transpose(1, 0, 2).reshape(128, NLAB * C)
    labsid = np.zeros((128, NLAB * C + 32), dtype=np.float32)
    labsid[:, : NLAB * C] = labp
    labsid[0:32, NLAB * C : NLAB * C + 32] = np.eye(32, dtype=np.float32)
    labsid = labsid.astype(bf16)

    counts = L.sum(axis=0)
    recip = np.where(counts != 0, 1.0 / np.maximum(counts, 1e-30), 0.0)
    recb = np.broadcast_to(recip.astype(np.float32), (BSH, C)).copy()

    shared = {
        "s2t": s2t, "sgnb": sgnb.astype(bf16), "sgw": sgw,
        "labsid": labsid, "recb": recb,
    }
    in_maps = []
    for c in range(NCORES):
        rows = slice(BSH * c, BSH * (c + 1))
        a2t_c = np.ascontiguousarray(
            a2[rows].T.reshape(DCH, 128, BSH))                     # [DCH,128,BSH]
        cc = np.zeros((2, N + BSH), dtype=np.float32)
        cc[0, :N] = 1.0
        cc[1, :N] = wS
        cc[0, N:] = kb - wa[rows]
        cc[1, N:] = 1.0
        in_maps.append(dict(shared, a2t=a2t_c, cc=cc))
    return in_maps


def kernel(**inputs) -> np.ndarray:
    from concourse.bass_utils import run_bass_kernel_spmd

    if "nc" not in _CACHE:
        _CACHE["nc"] = _build_nc()
    nc = _CACHE["nc"]

    in_maps = _prep_host(
        inputs["inputs"], inputs["support_tensors"], inputs["support_labels"],
        inputs["kernel_w"], inputs["kernel_b"],
    )
    res = run_bass_kernel_spmd(nc, in_maps, list(range(NCORES)))
    return np.concatenate([res.results[i]["out"] for i in range(NCORES)], axis=0)


# revision 38
# speedup vs baseline: 1.0579x; 1.0075x over previous
"""Trainium2 Bass kernel for the siamese-kNN classification head.

Reference computation (B=256, N=2000, D=512, C=100):
    scores[b,n] = sigmoid(sum_d w_d * |a[b,d] - S[n,d]| + kb)
    out[b,c]    = (scores @ L)[b,c] / count_c     (0 where count_c == 0)

Strategy
--------
Data-parallel over the batch: core i handles rows 32*i .. 32*i+32, no
collectives.  |x| = 2 relu(x) - x splits the score into a nonlinear slab
relu(A''-S'') (A''= alpha*2|w| (.) a, S''= alpha*2|w| (.) S, bf16) plus an
exact rank-2 f32r correction matmul (kb - w.a_b + (w.S)_n).

The B*N*D slab tensor is the whole cost.  Measured TRN2 engine rates for a
[128,2000] slab chunk:
  DVE tensor_scalar  bf16 out ~755ns   fp8e4 out ~1140ns
  ACT activation     any out  ~1860ns
  PE ingest          bf16 128 elem/cyc; fp8e4 DoubleRow 256 elem/cyc
so slabs are produced in three flavors, tuned so DVE/ACT/PE all finish
their ~79-80us of stream work together:
  - 21 mixed rows get one d-chunk-pair as bf16 DVE slabs (4x [128,500] mm)
  - the other pair of those rows + 11 full rows as alpha=64-scaled fp8
    pairs (DVE x22, ACT x21 units), consumed by dual-row fp8 matmuls
    ([128,2,500] moving k-tile-concatenated, dense aligned [128,2,32]
    weight windows -- walrus dual-fp8 ldweights rejects strided/odd-offset
    weights) at 2x ingest.  End-to-end rel err 1.53e-2 (fp8 quantization,
    bit-exact vs ml_dtypes emulation on host), under the 2e-2 gate.

Producer/PE instruction order comes from a trace-calibrated completion-time
simulation (_plan) so the in-order PE almost never stalls (measured <3us
idle).  PE warms up on dummy matmuls during the ~12us NEFF+DMA cold-start;
the first s2t chunk is DMA-split so the first slab starts earlier; small
constants ride the idle GpSimd DMA queue.  Tail: per-segment sigmoid (ACT)
from PSUM overlapped with the last matmuls, 16 PE transposes into one PSUM
bank, one copy, 16 bf16 label matmuls, 1/count scale (host-prepared
divide-no-nan), DMA out.  133.5us baseline -> 101.5us.
"""

import sys

for _p in ("/opt/trn_rl_repo", "/root/.axon_site/_ro/trn_rl_repo"):
    if _p not in sys.path:
        sys.path.append(_p)

import numpy as np

B, N, D, C = 256, 2000, 512, 100
NP = 2048                  # label rows padded to 16 full chunks
NCORES = 8
BSH = B // NCORES          # 32 batch rows per core
DCH = D // 128             # 4 d-chunks
NSEG = 4                   # PSUM free-dim segments
SEG = N // NSEG            # 500
HSEG = SEG // 2            # 250 (DoubleRow moving limit: 2*250 <= 512)
NLAB = NP // 128           # 16 label chunks
ALPHA = 64.0               # fp8 range pre-scale (exact power of 2)

F8_SPLIT = 1               # fp8 matmuls per psc segment (1 -> [128,2,500])
F8C = SEG // F8_SPLIT
KTAIL = 6                  # last units emitted segment-major (early psc stops)

# ---- producer assignment ----
# rows 0..20: mixed -- one chunk-pair bf16 (DVE), the other fp8.
#   even b: fp8 pair = 0; odd b: fp8 pair = 1.
# rows 21..31: both pairs fp8 (21-26 DVE, 27-31 ACT).
_MIX_DVE = frozenset(range(10))   # mixed rows whose fp8 pair runs on DVE
_FULL_DVE = (21, 22, 23, 24, 25, 26)
_FULL_ACT = (27, 28, 29, 30, 31)
N_MIXED = 21

_CACHE = {}


def _f8_units():
    """Canonical order of the fp8 (prod, pair, b) units; index = dense
    weight-window slot (dual-fp8 ldweights needs contiguous aligned
    [128,2,32] weights, so windows are materialized per unit)."""
    order = []
    for p in (0, 1):
        f8_rows = [b for b in range(N_MIXED) if (b % 2 == 0) == (p == 0)]
        for b in [x for x in f8_rows if x in _MIX_DVE] + list(_FULL_DVE):
            order.append(("dve", p, b))
        for b in [x for x in f8_rows if x not in _MIX_DVE] + list(_FULL_ACT):
            order.append(("act", p, b))
    return order


NF8 = len(_f8_units())


def _plan():
    """Static schedule: producer instruction order + PE consumption order.

    Items: dict(kind='bfs', ch, b) one bf16 chunk-slab, or
           dict(kind='f8p', pair, b, prod) one fp8 chunk-pair.
    Emission order of PE matmuls = estimated completion order, so the
    in-order PE never waits on a later-finishing producer while an
    earlier slab sits ready.
    """
    # trace-calibrated: first-DMA land / chunk spacing / engine first-instr
    CH_LAND = [11600.0, 13400.0, 14900.0, 16400.0]
    AB_LAND = 10400.0
    DVE_T0, ACT_T0 = 11600.0, 13500.0
    T_BF, T_F8H, T_AH = 755.0, 1140.0, 1860.0

    def interleave(la, lb):
        out, ia, ib = [], 0, 0
        while ia < len(la) or ib < len(lb):
            if ib >= len(lb) or (ia < len(la) and ia * len(lb) <= ib * len(la)):
                out.append(la[ia]); ia += 1
            else:
                out.append(lb[ib]); ib += 1
        return out

    items, dve_prog, act_prog = [], [], []
    for p in (0, 1):
        ch0, ch1 = 2 * p, 2 * p + 1
        bf_rows = [b for b in range(N_MIXED) if (b % 2 == 0) == (p == 1)]
        f8_rows = [b for b in range(N_MIXED) if (b % 2 == 0) == (p == 0)]
        f8_dve = [b for b in f8_rows if b in _MIX_DVE] + list(_FULL_DVE)
        f8_act = [b for b in f8_rows if b not in _MIX_DVE] + list(_FULL_ACT)

        s0 = [dict(kind="bfs", ch=ch0, b=b) for b in bf_rows]
        s1 = [dict(kind="bfs", ch=ch1, b=b) for b in bf_rows]
        fv = [dict(kind="f8p", pair=p, b=b, prod="dve") for b in f8_dve]
        fa = [dict(kind="f8p", pair=p, b=b, prod="act") for b in f8_act]
        items += s0 + s1 + fv + fa

        dve_prog += [("bfs", u) for u in s0]
        for entry in interleave([("bfs", u) for u in s1],
                                [("f8p", u) for u in fv]):
            if entry[0] == "bfs":
                dve_prog.append(entry)
            else:
                dve_prog.append(("f8h", entry[1], 0))
                dve_prog.append(("f8h", entry[1], 1))
        for u in fa:
            act_prog.append(("f8h", u, 0))
            act_prog.append(("f8h", u, 1))

    def sim(prog, t_bf, t_half, clock):
        for entry in prog:
            if entry[0] == "bfs":
                u = entry[1]
                clock = max(clock, CH_LAND[u["ch"]], AB_LAND) + t_bf
                u["done"] = clock
            else:
                _, u, h = entry
                clock = max(clock, CH_LAND[2 * u["pair"] + h], AB_LAND) + t_half
                if h == 1:
                    u["done"] = clock

    sim(dve_prog, T_BF, T_F8H, DVE_T0)
    sim(act_prog, 0.0, T_AH, ACT_T0)
    widx = {u: j for j, u in enumerate(_f8_units())}
    for u in items:
        if u["kind"] == "f8p":
            u["widx"] = widx[(u["prod"], u["pair"], u["b"])]
    pe_order = sorted(items, key=lambda u: u["done"])
    assert pe_order[0]["kind"] == "bfs" and pe_order[0]["ch"] == 0
    return dve_prog, act_prog, pe_order


def _split_multi_waits(nc):
    """TRN2 TPB instructions encode at most ONE semaphore wait; split extras
    into single-wait NOPs directly before the instruction (same engine)."""
    from concourse import mybir

    for fn in nc.m.functions:
        for bb in fn.blocks:
            out = []
            for inst in bb.instructions:
                si = inst.sync_info
                if si is not None and si.on_wait and len(si.on_wait) > 1:
                    waits = list(si.on_wait)
                    for j, w in enumerate(waits[:-1]):
                        out.append(mybir.InstNoOp(
                            name=f"{inst.name}-sw{j}", engine=inst.engine,
                            sync_info=mybir.SyncInfo(on_wait=[w], on_update=[]),
                            ins=[], outs=[]))
                    inst.sync_info = mybir.SyncInfo(
                        on_wait=[waits[-1]], on_update=list(si.on_update))
                out.append(inst)
            bb.instructions = out


def _build_nc():
    import concourse.bass as bass
    import concourse.tile as tile
    from concourse import mybir

    f32 = mybir.dt.float32
    f32r = mybir.dt.float32r
    bf16 = mybir.dt.bfloat16
    f8 = mybir.dt.float8e4
    nc = bass.Bass()

    s2t_d = nc.declare_dram_parameter("s2t", [D, N], bf16, isOutput=False)
    a2t_d = nc.declare_dram_parameter("a2t", [DCH, 128, BSH], f32, isOutput=False)
    sgnb_d = nc.declare_dram_parameter("sgnb", [128, DCH, 63], bf16, isOutput=False)
    sgw_d = nc.declare_dram_parameter("sgw", [128, NF8, 2, 32], f8, isOutput=False)
    cc_d = nc.declare_dram_parameter("cc", [2, N + BSH], f32r, isOutput=False)
    labsid_d = nc.declare_dram_parameter("labsid", [128, NLAB * C + 32],
                                         bf16, isOutput=False)
    recb_d = nc.declare_dram_parameter("recb", [BSH, C], f32, isOutput=False)
    out_d = nc.declare_dram_parameter("out", [BSH, C], f32, isOutput=True)

    dve_prog, act_prog, pe_order = _plan()
    sub = None  # set after mybir import below

    with tile.TileContext(nc) as tc:
        with (
            tc.tile_pool(name="const", bufs=1) as const,
            tc.tile_pool(name="dslab", bufs=6) as dpool,
            tc.tile_pool(name="vpair", bufs=4) as vpool,
            tc.tile_pool(name="apair", bufs=4) as apool,
            tc.tile_pool(name="bank", bufs=8, space="PSUM") as bankp,
        ):
            # ---- DMAs: s2t chunks (big, latency-critical) serial on the
            # Sync queue -- first chunk split in halves so the first slab
            # starts ~1us earlier.  Small constants go on the (otherwise
            # idle) GpSimd queue in parallel; big late-needed tensors (sgw,
            # labsid) follow the chunks on Sync to keep the first-chunk
            # window free of bandwidth contention.
            a2t = const.tile([128, DCH * BSH], f32, name="a2t", tag="a2t")
            nc.sync.dma_start(
                a2t[:].rearrange("p (c b) -> p c b", c=DCH),
                a2t_d[:].rearrange("c p b -> p c b"),
            )
            s2t0 = const.tile([128, N], bf16, name="s2t0", tag="s2t0")
            nc.sync.dma_start(s2t0[:, 0 : N // 2], s2t_d[0:128, 0 : N // 2])
            sgnb = const.tile([128, DCH, 63], bf16, name="sgnb", tag="sgnb")
            nc.sync.dma_start(sgnb[:], sgnb_d[:])
            nc.sync.dma_start(s2t0[:, N // 2 : N], s2t_d[0:128, N // 2 : N])
            s2t123 = const.tile([128, 3, N], bf16, name="s2t123", tag="s2t123")
            nc.sync.dma_start(s2t123[:, 0, :], s2t_d[128:256, :])
            nc.sync.dma_start(s2t123[:, 1, :], s2t_d[256:384, :])
            nc.sync.dma_start(s2t123[:, 2, :], s2t_d[384:512, :])
            sgw = const.tile([128, NF8, 2, 32], f8, name="sgw", tag="sgw")
            nc.sync.dma_start(sgw[:], sgw_d[:])
            labsid = const.tile([128, NLAB * C + 32], bf16,
                               name="labsid", tag="labsid")
            nc.sync.dma_start(labsid[:], labsid_d[:])
            cc = const.tile([2, N + BSH], f32r, name="cc", tag="cc")
            nc.gpsimd.dma_start(cc[:], cc_d[:])
            recb = const.tile([BSH, C], f32, name="recb", tag="recb")
            nc.gpsimd.dma_start(recb[:], recb_d[:])

            s2t = [s2t0] + [s2t123[:, k, :] for k in range(3)]
            ident = labsid[0:32, NLAB * C : NLAB * C + 32]

            sub_op = mybir.AluOpType.subtract
            min_op = mybir.AluOpType.min
            relu = mybir.ActivationFunctionType.Relu

            psc = [
                bankp.tile([BSH, SEG], f32, name=f"psc{s}", tag="bank")
                for s in range(NSEG)
            ]

            # ---- PE p-state warmup while DMAs run
            dummy_sb = const.tile([128, 512], bf16, name="dummy", tag="dummy")
            nc.vector.memset(dummy_sb[:], 0.0)
            dummy_ps = bankp.tile([2, 512], f32, name="dummy_ps", tag="bank")
            for _ in range(15):
                nc.tensor.matmul(
                    dummy_ps[:], dummy_sb[:, 0:2], dummy_sb[:],
                    start=True, stop=True, skip_group_check=True,
                )

            # ---- producers (DVE / ACT program order from the plan)
            n_bfs_seen = 0
            for entry in dve_prog:
                if entry[0] == "bfs":
                    u = entry[1]
                    slab = dpool.tile([128, N], bf16, name="dslab", tag="dslab")
                    col = u["ch"] * BSH + u["b"]
                    if n_bfs_seen < 2:
                        # first slabs chase the split s2t0 halves
                        for lo, hi in ((0, N // 2), (N // 2, N)):
                            nc.vector.tensor_scalar(
                                slab[:, lo:hi], s2t[u["ch"]][:, lo:hi],
                                a2t[:, col : col + 1], 0.0, sub_op, min_op,
                            )
                    else:
                        nc.vector.tensor_scalar(
                            slab[:], s2t[u["ch"]], a2t[:, col : col + 1],
                            0.0, sub_op, min_op,
                        )
                    n_bfs_seen += 1
                    u["ap"] = slab
                else:
                    _, u, h = entry
                    if h == 0:
                        u["ap"] = vpool.tile([128, NSEG * F8_SPLIT, 2, F8C], f8,
                                             name="vpair", tag="vpair")
                    ch = 2 * u["pair"] + h
                    nc.vector.tensor_scalar(
                        u["ap"][:, :, h, :], s2t[ch],
                        a2t[:, ch * BSH + u["b"] : ch * BSH + u["b"] + 1],
                        0.0, sub_op, min_op,
                    )
            for entry in act_prog:
                _, u, h = entry
                if h == 0:
                    u["ap"] = apool.tile([128, NSEG * F8_SPLIT, 2, F8C], f8,
                                         name="apair", tag="apair")
                ch = 2 * u["pair"] + h
                nc.scalar.activation(
                    u["ap"][:, :, h, :], s2t[ch], relu,
                    bias=a2t[:, ch * BSH + u["b"] : ch * BSH + u["b"] + 1],
                    scale=-1.0,
                )

            # ---- PE stream in estimated completion order
            def unit_seg_mms(u, s, first, stop):
                b = u["b"]
                if u["kind"] == "bfs":
                    nc.tensor.matmul(
                        psc[s][:], sgnb[:, u["ch"], 31 - b : 63 - b],
                        u["ap"][:, SEG * s : SEG * (s + 1)],
                        start=first, stop=stop,
                        skip_group_check=not first,
                    )
                else:
                    for j in range(F8_SPLIT):
                        nc.tensor.matmul(
                            psc[s][:, F8C * j : F8C * (j + 1)],
                            sgw[:, u["widx"], :, :],
                            u["ap"][:, F8_SPLIT * s + j, :, :],
                            start=False, stop=(stop and j == F8_SPLIT - 1),
                            perf_mode=mybir.MatmulPerfMode.DoubleRow,
                            skip_group_check=True,
                        )

            last_idx = len(pe_order) - 1
            for idx, u in enumerate(pe_order):
                for s in range(NSEG):
                    unit_seg_mms(u, s, first=(idx == 0), stop=(idx == last_idx))
                if idx == 6:
                    # exact rank-2 correction: kb - w.a_b + (w.S)_n
                    for s in range(NSEG):
                        nc.tensor.matmul(
                            psc[s][:], cc[:, N : N + BSH],
                            cc[:, SEG * s : SEG * (s + 1)],
                            start=False, stop=False, skip_group_check=True,
                        )

            # ---- sigmoid (PSUM -> SBUF, bf16) ----
            ssig = const.tile([BSH, N], bf16, name="ssig", tag="ssig")
            for s in range(NSEG):
                nc.scalar.activation(
                    ssig[:, SEG * s : SEG * (s + 1)], psc[s][:],
                    mybir.ActivationFunctionType.Sigmoid,
                )

            # ---- transposes into ONE PSUM bank, quartet-pipelined copy +
            # label matmuls (copy chunk j frees transposes 4j..4j+3)
            tpall = bankp.tile([128, NLAB * BSH], bf16, name="tpall", tag="bank")
            sct = const.tile([128, NLAB * BSH], bf16, name="sct", tag="sct")
            out_ps = bankp.tile([BSH, C], f32, name="out_ps", tag="bank")
            for k in range(NLAB):
                pk = min(128, N - 128 * k)
                nc.tensor.transpose(
                    tpall[:pk, BSH * k : BSH * k + BSH],
                    ssig[:, 128 * k : 128 * k + pk], ident,
                )
            # 2-way copy so the first 8 label matmuls overlap the 2nd copy
            nc.vector.tensor_copy(sct[:, : BSH * 8], tpall[:, : BSH * 8])
            nc.vector.tensor_copy(sct[:, BSH * 8 :], tpall[:, BSH * 8 :])
            for k in range(NLAB):
                pk = min(128, N - 128 * k)
                nc.tensor.matmul(
                    out_ps[:], sct[:pk, BSH * k : BSH * k + BSH],
                    labsid[:pk, C * k : C * (k + 1)],
                    start=(k == 0), stop=(k == NLAB - 1),
                )

            # ---- divide by counts, write out ----
            out_s = const.tile([BSH, C], f32, name="out_s", tag="out_s")
            nc.vector.tensor_mul(out_s[:], out_ps[:], recb[:])
            nc.sync.dma_start(out_d[:], out_s[:])

    _split_multi_waits(nc)
    return nc


def _prep_host(inputs, support_tensors, support_labels, kernel_w, kernel_b):
    import ml_dtypes

    bf16 = ml_dtypes.bfloat16
    f8 = ml_dtypes.float8_e4m3
    a = np.asarray(inputs, dtype=np.float32)
    S = np.asarray(support_tensors, dtype=np.float32)
    L = np.asarray(support_labels, dtype=np.float32)
    w = np.asarray(kernel_w, dtype=np.float32)
    kb = np.float32(np.asarray(kernel_b, dtype=np.float32))

    aw = ALPHA * 2.0 * np.abs(w)
    sgn = np.sign(w).astype(np.float32)
    s2t = np.ascontiguousarray((S * aw[None, :]).T).astype(bf16)   # [D, N]
    wS = (S @ w).astype(np.float32)                                # [N]
    wa = (a @ w).astype(np.float32)                                # [B]
    a2 = a * aw[None, :]                                           # [B, D]

    sgn_chunks = sgn.reshape(DCH, 128).T                           # [128, DCH]
    # bf16 sliding-window sign tiles (negative slabs): col 31 = -sgn/alpha
    sgnb = np.zeros((128, DCH, 63), dtype=np.float32)
    sgnb[:, :, 31] = -sgn_chunks / ALPHA
    # dense fp8 weight windows, one [128,2,32] slot per fp8 unit
    sgw = np.zeros((128, NF8, 2, 32), dtype=np.float32)
    for j, (prod, p, b) in enumerate(_f8_units()):
        pol = -1.0 if prod == "dve" else 1.0
        for i in range(2):
            sgw[:, j, i, b] = pol * sgn_chunks[:, 2 * p + i] / ALPHA
    sgw = sgw.astype(f8)

    labp = np.zeros((NP, C), dtype=np.float32)
    labp[:N] = L
    labp = labp.reshape(NLAB, 128, C).transpose(1, 0, 2).reshape(128, NLAB * C)
    labsid = np.zeros((128, NLAB * C + 32), dtype=np.float32)
    labsid[:, : NLAB * C] = labp
    labsid[0:32, NLAB * C : NLAB * C + 32] = np.eye(32, dtype=np.float32)
    labsid = labsid.astype(bf16)

    counts = L.sum(axis=0)
    recip = np.where(counts != 0, 1.0 / np.maximum(counts, 1e-30), 0.0)
    recb = np.broadcast_to(recip.astype(np.float32), (BSH, C)).copy()

    shared = {
        "s2t": s2t, "sgnb": sgnb.astype(bf16), "sgw": sgw,
        "labsid": labsid, "recb": recb,
    }
    in_maps = []
    for c in range(NCORES):
        rows = slice(BSH * c, BSH * (c + 1))
        a2t_c = np.ascontiguousarray(
            a2[rows].T.reshape(DCH, 128, BSH))                     # [DCH,128,BSH]
        cc = np.zeros((2, N + BSH), dtype=np.float32)
        cc[0, :N] = 1.0
        cc[1, :N] = wS
        cc[0, N:] = kb - wa[rows]
        cc[1, N:] = 1.0
        in_maps.append(dict(shared, a2t=a2t_c, cc=cc))
    return in_maps


def kernel(**inputs) -> np.ndarray:
    from concourse.bass_utils import run_bass_kernel_spmd

    if "nc" not in _CACHE:
        _CACHE["nc"] = _build_nc()
    nc = _CACHE["nc"]

    in_maps = _prep_host(
        inputs["inputs"], inputs["support_tensors"], inputs["support_labels"],
        inputs["kernel_w"], inputs["kernel_b"],
    )
    res = run_bass_kernel_spmd(nc, in_maps, list(range(NCORES)))
    return np.concatenate([res.results[i]["out"] for i in range(NCORES)], axis=0)
